# revision 36
# baseline (speedup 1.0000x reference)
"""AgentHetGNN layer on 8 Trainium2 NeuronCores (Bass/Tile, SPMD data-parallel).

Strategy
--------
Data-parallel over the 64 scenes: 8 scenes per core.  Inside a core,
per scene:
  A) LayerNorm (stats on DVE, apply fused sub+mul) + projections.
     Activations move feature-major ("^T", [feat, token]) so matmul
     contractions land on partitions; per-128-token tiles are PE-transposed.
     LN gammas are folded into the projection weights on the host, the
     attention scale is folded into wq, v-projection biases are folded
     into the post-attention fc biases (all exact algebra).
  B) Attention per (relation, head, key-tile): scores are produced
     TRANSPOSED (s^T = k_h^T.T @ q_h^T, K=32 contraction), exp on ACT
     (PSUM->SBUF, bf16), multiplicative mask (notmask, bf16, DVE 4x mode),
     then o = p^T.T @ [v | 1] with a ones-column interleaved into v so the
     softmax denominator accumulates for free in the same PSUM tile.
     No max-subtraction is needed: scores are O(1) by construction and
     masked entries are exactly zeroed after exp.
  C) fc/out_fc/FFN batched across all 8 scenes per core (moving operand
     width 512-1024), silu on ACT, fused (psum+bias)*t / (psum+bias)+h
     on DVE via scalar_tensor_tensor.

All heavy matmuls run in bf16 (1 cyc/row on PE); f32 is kept for inputs,
LN statistics, PSUM accumulation, the residual h, and the final output.
"""

import os
import numpy as np

import concourse.bass as bass
import concourse.mybir as mybir
import concourse.tile as tile
from concourse.bass_utils import run_bass_kernel_spmd
from concourse.masks import make_identity

F32 = mybir.dt.float32
BF16 = mybir.dt.bfloat16
AF = mybir.ActivationFunctionType
ALU = mybir.AluOpType

N_CORES = 8
B = 64
B_LOC = B // N_CORES          # scenes per core
NA, NL, NG, D = 128, 1024, 512, 128
H, DK = 4, 32
P = 128
EPS = 1e-5

# relations: (name, n_key_tokens, tiles-of-128)
RELS = [("a2a", NA, NA // P), ("l2a", NL, NL // P), ("g2a", NG, NG // P)]

# ---------------------------------------------------------------------------
# weight packing offsets (columns in the single packed bf16 weight tensor)
# layout: wq(384) | k_o(128) v_o(128) | k_l(128) v_l(128) | k_g(128) v_g(128)
#         | wfc(4*128) | wout(4*128) | w1(512) | w3(512) | w2(4*128)
_OFF = {}
_c = 0
for _name, _w in [("wq", 384), ("kv_o", 256), ("kv_l", 256), ("kv_g", 256),
                  ("wfc", 512), ("wout", 512), ("w1", 512), ("w3", 512),
                  ("w2", 512)]:
    _OFF[_name] = _c
    _c += _w
W_COLS = _c

# bias packing: bq(3) | bfc(4) | bout(1) | b1(4) | b3(4) | b2(1)
_BOFF = {"bq": 0, "bfc": 3, "bout": 7, "b1": 8, "b3": 12, "b2": 16}
B_COLS = 17


def _build_program():
    from concourse import bacc
    nc = bacc.Bacc(None)

    agent = nc.declare_dram_parameter("agent", [B_LOC, NA, D], F32, isOutput=False)
    lane = nc.declare_dram_parameter("lane", [B_LOC, NL, D], F32, isOutput=False)
    poly = nc.declare_dram_parameter("poly", [B_LOC, NG, D], F32, isOutput=False)
    nm_a = nc.declare_dram_parameter("nm_a", [B_LOC, P, (NA // P) * NA], BF16, isOutput=False)
    nm_l = nc.declare_dram_parameter("nm_l", [B_LOC, P, (NL // P) * NA], BF16, isOutput=False)
    nm_g = nc.declare_dram_parameter("nm_g", [B_LOC, P, (NG // P) * NA], BF16, isOutput=False)
    wpack = nc.declare_dram_parameter("wpack", [P, W_COLS], BF16, isOutput=False)
    bpack = nc.declare_dram_parameter("bpack", [P, B_COLS], F32, isOutput=False)
    out = nc.declare_dram_parameter("out", [B_LOC, NA, D], F32, isOutput=True)

    srcs = {"a2a": agent, "l2a": lane, "g2a": poly}
    nms = {"a2a": nm_a, "l2a": nm_l, "g2a": nm_g}

    with tile.TileContext(nc) as tc:
        from contextlib import ExitStack
        with ExitStack() as ctx:
            cpool = ctx.enter_context(tc.tile_pool(name="const", bufs=1))
            inp = ctx.enter_context(tc.tile_pool(name="inp", bufs=3))
            act = ctx.enter_context(tc.tile_pool(name="act", bufs=3))
            small = ctx.enter_context(tc.tile_pool(name="small", bufs=4))
            ptp = ctx.enter_context(tc.tile_pool(name="ptp", bufs=4))
            core = ctx.enter_context(tc.tile_pool(name="core", bufs=1))
            mm = ctx.enter_context(tc.tile_pool(name="mm", bufs=3, space="PSUM"))
            pop = ctx.enter_context(tc.tile_pool(name="pop", bufs=2, space="PSUM"))
            ptq = ctx.enter_context(tc.tile_pool(name="ptq", bufs=3, space="PSUM"))

            # ---- constants -------------------------------------------------
            w_sb = cpool.tile([P, W_COLS], BF16)
            nc.sync.dma_start(w_sb[:], wpack[:])
            b_sb = cpool.tile([P, B_COLS], F32)
            nc.sync.dma_start(b_sb[:], bpack[:])
            id_bf = cpool.tile([P, P], BF16)
            make_identity(nc, id_bf[:])
            id_f32 = cpool.tile([P, P], F32)
            make_identity(nc, id_f32[:])
            eps_sb = cpool.tile([P, 1], F32)
            nc.vector.memset(eps_sb[:], EPS)
            sim_compat = bool(int(os.environ.get("KERNEL_SIM_COMPAT", "0")))

            def wslice(name, c0, n):
                return w_sb[:, _OFF[name] + c0:_OFF[name] + c0 + n]

            def bcol(name, j=0):
                return b_sb[:, _BOFF[name] + j:_BOFF[name] + j + 1]

            # per-relation, per-core-level activation buffers
            oT_all = {r: core.tile([P, B_LOC * P], BF16, tag=f"oT_{r}",
                                   name=f"oT_{r}")
                      for r, _, _ in RELS}
            agT_all = core.tile([P, B_LOC * P], BF16, tag="agT")
            hT_all = core.tile([P, B_LOC * P], F32, tag="hT")
            hnT_all = core.tile([P, B_LOC * P], BF16, tag="hnT")
            g_all = core.tile([P, 4, B_LOC * P], BF16, tag="g")
            outT_all = core.tile([P, B_LOC * P], F32, tag="outT")

            def ln_stats(mv, st6, x_sb, T, base):
                """bn stats for T tiles of x_sb into mv[:, base:base+T, :]."""
                for t in range(T):
                    nc.vector.bn_stats(st6[:, base + t, :], x_sb[:, t, :])
                    nc.vector.bn_aggr(mv[:, base + t, :], st6[:, base + t, :])

            def ln_rstd(rstd, mv, n):
                """rstd[:, :n] = 1/sqrt(var+eps), one batched ACT pass.

                On HW: exp(-0.5*ln(var+eps)) keeps the ACT engine in the
                natural_log_exp table (no table-load thrash against the
                attention exps).  CoreSim lacks Ln, so a sim-compat build
                uses Sqrt + DVE reciprocal instead."""
                if sim_compat:
                    nc.scalar.activation(rstd[:, :n], mv[:, :n, 1], AF.Sqrt,
                                         bias=eps_sb[:])
                    nc.vector.reciprocal(rstd[:, :n], rstd[:, :n])
                else:
                    nc.scalar.activation(rstd[:, :n], mv[:, :n, 1], AF.Ln,
                                         bias=eps_sb[:])
                    nc.scalar.activation(rstd[:, :n], rstd[:, :n], AF.Exp,
                                         scale=-0.5)

            def ln_apply(xn_dst, x_sb, mv, rstd, T, base):
                # apply on the (otherwise idle) Pool engine: 1-tensor-input op
                for t in range(T):
                    nc.gpsimd.tensor_scalar(
                        xn_dst[:, t, :], x_sb[:, t, :],
                        mv[:, base + t, 0:1], rstd[:, base + t:base + t + 1],
                        ALU.subtract, ALU.mult)

            _evac_flip = [0]

            def transpose_128(dst_sb, src_sb, ident, dtype, evac=None):
                """dst_sb[128,128] = src_sb[128,128].T via PE (through PSUM).

                The PSUM evacuation copy alternates DVE / ACT for balance
                unless an engine is forced via `evac`.
                """
                pt = ptq.tile([P, P], dtype, tag="pt")
                nc.tensor.transpose(pt[:], src_sb[:], ident)
                if evac is None:
                    _evac_flip[0] ^= 1
                    evac = nc.vector if _evac_flip[0] else nc.scalar
                if evac is nc.scalar:
                    nc.scalar.copy(dst_sb, pt[:])
                else:
                    nc.vector.tensor_copy(dst_sb, pt[:])

            N_TILES = sum(T for _, _, T in RELS)   # 13
            for s in range(B_LOC):
                # ---- phase A: LN + projections -----------------------------
                xnT = {}
                v_aug = {}
                kT = {}
                qT = act.tile([P, 3, P], BF16, tag="qT")
                # DMA + LN stats for all sources, one batched rstd pass
                x_sbs = {}
                st6 = small.tile([P, N_TILES, 6], F32, tag="st6")
                mv = small.tile([P, N_TILES, 2], F32, tag="mv")
                rstd = small.tile([P, N_TILES], F32, tag="rstd")
                base = 0
                bases = {}
                for r, NK, T in RELS:
                    x_sb = inp.tile([P, T, P], F32, tag=f"x_{r}")
                    nc.sync.dma_start(
                        x_sb[:], srcs[r][s].rearrange("(t p) d -> p t d", p=P))
                    x_sbs[r] = x_sb
                    ln_stats(mv, st6, x_sb, T, base)
                    bases[r] = base
                    base += T
                ln_rstd(rstd, mv, N_TILES)
                for r, NK, T in RELS:
                    x_sb = x_sbs[r]
                    xn = act.tile([P, T, P], BF16, tag=f"xn_{r}")
                    ln_apply(xn, x_sb, mv, rstd, T, bases[r])
                    # transpose each token tile -> xnT (feature-major)
                    xT = act.tile([P, T, P], BF16, tag=f"xT_{r}")
                    for t in range(T):
                        transpose_128(xT[:, t, :], xn[:, t, :], id_bf[:], BF16)
                    xnT[r] = xT
                    # k^T = Wk.T @ xn^T   (all tiles in <=512 chunks)
                    kr = act.tile([P, T * P], BF16, tag=f"kT_{r}")
                    kvkey = {"a2a": "kv_o", "l2a": "kv_l", "g2a": "kv_g"}[r]
                    wk = wslice(kvkey, 0, P)
                    for c0 in range(0, T * P, 512):
                        cw = min(512, T * P - c0)
                        pk = mm.tile([P, 512], F32, tag="mm")
                        nc.tensor.matmul(
                            pk[:, :cw], wk,
                            xT.rearrange("p t d -> p (t d)")[:, c0:c0 + cw])
                        nc.vector.tensor_copy(kr[:, c0:c0 + cw], pk[:, :cw])
                    kT[r] = kr
                    # v = xn @ Wv, ones column interleaved: v_aug[128,T,4,33]
                    wv = wslice(kvkey, P, P)
                    va = act.tile([P, T, 4, 33], BF16, tag=f"va_{r}")
                    # ones columns for the fused softmax denominator (same-size
                    # bitcast, so the strided AP is fine for memset)
                    nc.gpsimd.memset(va[:, :, :, 32:33], 1.0)
                    for t in range(T):
                        pv = mm.tile([P, 512], F32, tag="mm")
                        nc.tensor.matmul(pv[:, :P], xT[:, t, :], wv)
                        vsrc = pv[:, :P].rearrange("p (h d) -> p h d", h=H)
                        if t % 2 == 0:
                            nc.scalar.copy(va[:, t, :, 0:32], vsrc)
                        else:
                            nc.vector.tensor_copy(va[:, t, :, 0:32], vsrc)
                    v_aug[r] = va
                    if r == "a2a":
                        # q^T (3 relation blocks) from the agent xn^T
                        for j in range(3):
                            pq = mm.tile([P, 512], F32, tag="mm")
                            nc.tensor.matmul(pq[:, :P], wslice("wq", j * P, P),
                                             xT[:, 0, :])
                            nc.vector.tensor_scalar(
                                qT[:, j, :], pq[:, :P], bcol("bq", j), None,
                                ALU.add)
                        # raw agent^T (bf16) for self_fc
                        xraw = act.tile([P, P], BF16, tag="xraw")
                        nc.vector.tensor_copy(xraw[:], x_sb[:, 0, :])
                        transpose_128(agT_all[:, s * P:(s + 1) * P], xraw[:],
                                      id_bf[:], BF16)

                # ---- phase B: attention ------------------------------------
                for ri, (r, NK, T) in enumerate(RELS):
                    nm_sb = act.tile([P, T, P], BF16, tag=f"nm_{r}")
                    nc.sync.dma_start(
                        nm_sb[:], nms[r][s].rearrange("p (t q) -> p t q", t=T))
                    po = pop.tile([P, H * 33], F32, tag="po")
                    kT3 = kT[r].rearrange("p (t d) -> p t d", d=P)
                    for h in range(H):
                        hs = slice(h * DK, (h + 1) * DK)
                        # all score tiles for this head first (keeps the PE in
                        # 32-row tiling mode), then all o-matmuls (full mode)
                        pt = ptp.tile([P, 8, P], BF16, tag="pT", name="pT")
                        for t0 in range(0, T, 4):
                            g = min(4, T - t0)
                            ps = mm.tile([P, 4, P], F32, tag="mm", name="ps")
                            for j in range(g):
                                nc.tensor.matmul(
                                    ps[:, j, :], kT3[hs, t0 + j, :],
                                    qT[hs, ri, :],
                                    tile_position=(h * DK, 0))
                            nc.scalar.activation(pt[:, t0:t0 + g, :],
                                                 ps[:, :g, :], AF.Exp)
                            # mask multiply: mostly DVE (bf16 2x), 1/4 to Pool
                            eng = nc.gpsimd if (t0 // 4 + h) % 4 == 3 else nc.vector
                            eng.tensor_mul(pt[:, t0:t0 + g, :],
                                           pt[:, t0:t0 + g, :],
                                           nm_sb[:, t0:t0 + g, :])
                        for t in range(T):
                            nc.tensor.matmul(
                                po[:, h * 33:(h + 1) * 33],
                                pt[:, t, :],
                                v_aug[r][:, t, h, :],
                                start=(t == 0), stop=(t == T - 1))
                    # normalize + evacuate o, then transpose to oT_all
                    po3 = po.rearrange("p (h c) -> p h c", c=33)
                    rcp = small.tile([P, H, 1], F32, tag="rcp")
                    nc.vector.reciprocal(rcp[:, :, 0], po3[:, :, 32])
                    o_sb = act.tile([P, P], BF16, tag="o_sb")
                    nc.vector.tensor_tensor(
                        o_sb.rearrange("p (h d) -> p h d", h=H),
                        po3[:, :, 0:32],
                        rcp[:].to_broadcast((P, H, DK)),
                        ALU.mult)
                    transpose_128(oT_all[r][:, s * P:(s + 1) * P], o_sb[:],
                                  id_bf[:], BF16)

            # ---- phase C: fc / out_fc / FFN over all scenes ----------------
            QW = B_LOC * P     # 1024 moving width
            fc_in = [agT_all, oT_all["a2a"], oT_all["l2a"], oT_all["g2a"]]
            # cat order in reference: [h_self, h_other, h_l2a, h_g2a]
            hT = [core.tile([P, QW], BF16, tag=f"hT_{j}", name=f"hT_{j}")
                  for j in range(4)]
            for j in range(4):
                for c0 in range(0, QW, 512):
                    pf = mm.tile([P, 512], F32, tag="mm")
                    nc.tensor.matmul(pf[:], wslice("wfc", j * P, P),
                                     fc_in[j][:, c0:c0 + 512])
                    nc.scalar.activation(hT[j][:, c0:c0 + 512], pf[:],
                                         AF.Relu, bias=bcol("bfc", j))
            for c0 in range(0, QW, 512):
                ph = mm.tile([P, 512], F32, tag="mm")
                for j in range(4):
                    nc.tensor.matmul(ph[:], wslice("wout", j * P, P),
                                     hT[j][:, c0:c0 + 512],
                                     start=(j == 0), stop=(j == 3))
                nc.vector.tensor_scalar(hT_all[:, c0:c0 + 512], ph[:],
                                        bcol("bout"), None, ALU.add)
            # FFN layernorm (token-major transpose round trip); stats for all
            # scenes first so the rstd ACT pass is batched once
            h_tok = core.tile([P, B_LOC, P], F32, tag="h_tok")
            st6f = small.tile([P, B_LOC, 6], F32, tag="st6")
            mvf = small.tile([P, B_LOC, 2], F32, tag="mv")
            rstdf = small.tile([P, B_LOC], F32, tag="rstd")
            for s in range(B_LOC):
                transpose_128(h_tok[:, s, :], hT_all[:, s * P:(s + 1) * P],
                              id_f32[:], F32)
                ln_stats(mvf, st6f, h_tok[:, s:s + 1, :], 1, s)
            ln_rstd(rstdf, mvf, B_LOC)
            for s in range(B_LOC):
                hn = act.tile([P, P], BF16, tag="hn_tok")
                ln_apply(hn.rearrange("p (t d) -> p t d", t=1),
                         h_tok[:, s:s + 1, :], mvf, rstdf, 1, s)
                transpose_128(hnT_all[:, s * P:(s + 1) * P], hn[:],
                              id_bf[:], BF16)
            # w1 (silu) * w3 -> g ; w2 @ g + b2 + h -> outT
            for c0 in range(0, QW, 512):
                for c in range(4):
                    p1 = mm.tile([P, 512], F32, tag="mm")
                    nc.tensor.matmul(p1[:], wslice("w1", c * P, P),
                                     hnT_all[:, c0:c0 + 512])
                    # silu(a) = a * sigmoid(a): sigmoid on ACT, multiply fused
                    # into the PSUM evacuation (CoreSim lacks a Silu table)
                    sg = act.tile([P, 512], BF16, tag="sg")
                    nc.scalar.activation(sg[:], p1[:], AF.Sigmoid,
                                         bias=bcol("b1", c))
                    t1 = act.tile([P, 512], BF16, tag="t1")
                    nc.vector.scalar_tensor_tensor(
                        t1[:], p1[:], bcol("b1", c), sg[:], ALU.add, ALU.mult)
                    p3 = mm.tile([P, 512], F32, tag="mm")
                    nc.tensor.matmul(p3[:], wslice("w3", c * P, P),
                                     hnT_all[:, c0:c0 + 512])
                    nc.vector.scalar_tensor_tensor(
                        g_all[:, c, c0:c0 + 512], p3[:], bcol("b3", c), t1[:],
                        ALU.add, ALU.mult)
                pf = mm.tile([P, 512], F32, tag="mm")
                for c in range(4):
                    nc.tensor.matmul(pf[:], wslice("w2", c * P, P),
                                     g_all[:, c, c0:c0 + 512],
                                     start=(c == 0), stop=(c == 3))
                nc.vector.scalar_tensor_tensor(
                    outT_all[:, c0:c0 + 512], pf[:], bcol("b2"),
                    hT_all[:, c0:c0 + 512], ALU.add, ALU.add)
            for s in range(B_LOC):
                o_tok = act.tile([P, P], F32, tag="o_tok")
                transpose_128(o_tok[:], outT_all[:, s * P:(s + 1) * P],
                              id_f32[:], F32)
                nc.sync.dma_start(out[s], o_tok[:])

    # Constrain the act-table chooser so Exp and Ln resolve to the ONE table
    # that holds both (natural_log_exp_and_others): without this, Exp picks
    # exp_and_others and Ln picks natural_log and every LN<->attention
    # transition pays a ~1.3us ACT table load (61 loads -> ~78us/core).
    # Table positions (= act_func_set ids walrus consumes) are unchanged;
    # we only shrink the competing sets the chooser may select.
    import concourse.bacc as bacc_mod
    _orig_tables = bacc_mod.get_activation_tables

    def _constrained(arch):
        t = {k: set(v) for k, v in _orig_tables(arch).items()}
        for name, s in t.items():
            if name != "natural_log_exp_and_others":
                s.discard(AF.Exp)
                s.discard(AF.Ln)
        return t

    bacc_mod.get_activation_tables = _constrained
    try:
        nc.compile()
    finally:
        bacc_mod.get_activation_tables = _orig_tables
    return nc


_PROG = None


def _get_program():
    global _PROG
    if _PROG is None:
        _PROG = _build_program()
    return _PROG


def _prep_host(params):
    """Fold LN affine params / attention scales / v-biases into weights.

    Returns (wpack[128, W_COLS] bf16, bpack[128, B_COLS] f32).
    """
    import math as _m

    def _np(v):
        if isinstance(v, (tuple, list)):
            return tuple(np.asarray(x, np.float32) for x in v)
        return np.asarray(v, np.float32)

    g = {k: _np(v) for k, v in params.items()}

    def fold(ln, w):
        gam, bet = ln
        return gam[:, None] * w, bet @ w

    wq, bq = fold(g["ln_q"], g["wq"])             # [128,384], [384]
    # attention scale folded into each relation's q block
    for j, nk in enumerate([NA, NL, NG]):
        sc = _m.log(nk + 1.0, 32) / _m.sqrt(DK)
        wq[:, j * P:(j + 1) * P] *= sc
        bq[j * P:(j + 1) * P] *= sc
    kv = {}
    vbias = {}
    for r, lnn, wn in [("o", "ln_other", "wkv_other"),
                       ("l", "ln_l2a", "wkv_l2a"),
                       ("g", "ln_g2a", "wkv_g2a")]:
        w, b = fold(g[lnn], g[wn])                # [128,256], [256]
        kv[r] = w
        kv[r + "_kb"] = b[:P]                      # k bias (zero for this model)
        vbias[r] = b[P:]                           # v bias -> folded into fc bias
    # fc weights / biases in device order (self, other, l2a, g2a), with the
    # v-bias fold: relu((o + vb) @ W + b) = relu(o @ W + (vb @ W + b))
    wfc = np.concatenate([g["self_fc"][0], g["fc_other"][0], g["fc_l2a"][0],
                          g["fc_g2a"][0]], axis=1)         # [128, 4*128]
    bfc = np.stack([
        g["self_fc"][1],
        g["fc_other"][1] + vbias["o"] @ g["fc_other"][0],
        g["fc_l2a"][1] + vbias["l"] @ g["fc_l2a"][0],
        g["fc_g2a"][1] + vbias["g"] @ g["fc_g2a"][0]], axis=1)
    # out_fc: cat order [self, other, l2a, g2a] rows of wout
    wout, bout = g["out_fc"]                       # [512,128],[128]
    wout_p = wout.reshape(4, P, P).transpose(1, 0, 2).reshape(P, 4 * P)
    # ffn
    w1, b1 = g["ffn_w1"]
    w2, b2 = g["ffn_w2"]
    w3, b3 = g["ffn_w3"]
    gam, bet = g["ffn_ln"]
    w1f = gam[:, None] * w1
    b1f = bet @ w1 + b1
    w3f = gam[:, None] * w3
    b3f = bet @ w3 + b3
    w2_p = w2.reshape(4, P, P).transpose(1, 0, 2).reshape(P, 4 * P)

    wpack = np.zeros((P, W_COLS), np.float32)
    wpack[:, _OFF["wq"]:_OFF["wq"] + 384] = wq
    for r, key in [("o", "kv_o"), ("l", "kv_l"), ("g", "kv_g")]:
        wpack[:, _OFF[key]:_OFF[key] + 256] = kv[r]
    wpack[:, _OFF["wfc"]:_OFF["wfc"] + 512] = wfc
    wpack[:, _OFF["wout"]:_OFF["wout"] + 512] = wout_p
    wpack[:, _OFF["w1"]:_OFF["w1"] + 512] = w1f
    wpack[:, _OFF["w3"]:_OFF["w3"] + 512] = w3f
    wpack[:, _OFF["w2"]:_OFF["w2"] + 512] = w2_p

    bpack = np.zeros((P, B_COLS), np.float32)
    bpack[:, _BOFF["bq"]:_BOFF["bq"] + 3] = bq.reshape(3, P).T
    bpack[:, _BOFF["bfc"]:_BOFF["bfc"] + 4] = bfc
    bpack[:, _BOFF["bout"]] = bout
    bpack[:, _BOFF["b1"]:_BOFF["b1"] + 4] = b1f.reshape(4, P).T
    bpack[:, _BOFF["b3"]:_BOFF["b3"] + 4] = b3f.reshape(4, P).T
    bpack[:, _BOFF["b2"]] = b2

    import ml_dtypes
    return wpack.astype(ml_dtypes.bfloat16), bpack


def _prep_mask(mask):
    """bool [Bl, NA, NK] -> notmask^T bf16 [Bl, 128, T*NA] (k-partition-major)."""
    import ml_dtypes
    bl, na, nk = mask.shape
    t = nk // P
    nmT = (~np.asarray(mask, bool)).astype(np.float32).transpose(0, 2, 1)
    nmT = nmT.reshape(bl, t, P, na).transpose(0, 2, 1, 3).reshape(bl, P, t * na)
    return np.ascontiguousarray(nmT.astype(ml_dtypes.bfloat16))


def kernel(agent_h, lane_h, poly_h, a2a_mask, l2a_mask, g2a_mask, params):
    agent_h = np.ascontiguousarray(np.asarray(agent_h, np.float32))
    lane_h = np.ascontiguousarray(np.asarray(lane_h, np.float32))
    poly_h = np.ascontiguousarray(np.asarray(poly_h, np.float32))
    wpack, bpack = _prep_host(params)
    nm_a = _prep_mask(np.asarray(a2a_mask))
    nm_l = _prep_mask(np.asarray(l2a_mask))
    nm_g = _prep_mask(np.asarray(g2a_mask))

    nc = _get_program()
    in_maps = []
    for c in range(N_CORES):
        sl = slice(c * B_LOC, (c + 1) * B_LOC)
        in_maps.append({
            "agent": agent_h[sl], "lane": lane_h[sl], "poly": poly_h[sl],
            "nm_a": nm_a[sl], "nm_l": nm_l[sl], "nm_g": nm_g[sl],
            "wpack": wpack, "bpack": bpack,
        })
    trace = bool(int(os.environ.get("KERNEL_TRACE", "0")))
    res = run_bass_kernel_spmd(nc, in_maps, list(range(N_CORES)), trace=trace)
    if trace and res.exec_time_ns is not None:
        print(f"HW exec time: {res.exec_time_ns} ns")
    out = np.concatenate([res.results[c]["out"] for c in range(N_CORES)], axis=0)
    return out.astype(np.float32)


# revision 37
# speedup vs baseline: 12416.2835x; 12416.2835x over previous
"""AgentHetGNN layer on 8 Trainium2 NeuronCores (Bass/Tile, SPMD data-parallel).

Strategy
--------
Data-parallel over the 64 scenes: 8 scenes per core.  Inside a core,
per scene:
  A) LayerNorm (stats on DVE, apply fused sub+mul) + projections.
     Activations move feature-major ("^T", [feat, token]) so matmul
     contractions land on partitions; per-128-token tiles are PE-transposed.
     LN gammas are folded into the projection weights on the host, the
     attention scale is folded into wq, v-projection biases are folded
     into the post-attention fc biases (all exact algebra).
  B) Attention per (relation, head, key-tile): scores are produced
     TRANSPOSED (s^T = k_h^T.T @ q_h^T, K=32 contraction), exp on ACT
     (PSUM->SBUF, bf16), multiplicative mask (notmask, bf16, DVE 4x mode),
     then o = p^T.T @ [v | 1] with a ones-column interleaved into v so the
     softmax denominator accumulates for free in the same PSUM tile.
     No max-subtraction is needed: scores are O(1) by construction and
     masked entries are exactly zeroed after exp.
  C) fc/out_fc/FFN batched across all 8 scenes per core (moving operand
     width 512-1024), silu on ACT, fused (psum+bias)*t / (psum+bias)+h
     on DVE via scalar_tensor_tensor.

All heavy matmuls run in bf16 (1 cyc/row on PE); f32 is kept for inputs,
LN statistics, PSUM accumulation, the residual h, and the final output.
"""

import os
import numpy as np

import concourse.bass as bass
import concourse.mybir as mybir
import concourse.tile as tile
from concourse.bass_utils import run_bass_kernel_spmd
from concourse.masks import make_identity

F32 = mybir.dt.float32
BF16 = mybir.dt.bfloat16
AF = mybir.ActivationFunctionType
ALU = mybir.AluOpType

N_CORES = 8
B = 64
B_LOC = B // N_CORES          # scenes per core
NA, NL, NG, D = 128, 1024, 512, 128
H, DK = 4, 32
P = 128
EPS = 1e-5

# relations: (name, n_key_tokens, tiles-of-128)
RELS = [("a2a", NA, NA // P), ("l2a", NL, NL // P), ("g2a", NG, NG // P)]

# ---------------------------------------------------------------------------
# weight packing offsets (columns in the single packed bf16 weight tensor)
# layout: wq(384) | k_o(128) v_o(128) | k_l(128) v_l(128) | k_g(128) v_g(128)
#         | wfc(4*128) | wout(4*128) | w1(512) | w3(512) | w2(4*128)
_OFF = {}
_c = 0
for _name, _w in [("wq", 384), ("kv_o", 256), ("kv_l", 256), ("kv_g", 256),
                  ("wfc", 512), ("wout", 512), ("w1", 512), ("w3", 512),
                  ("w2", 512)]:
    _OFF[_name] = _c
    _c += _w
W_COLS = _c

# bias packing: bq(3) | bfc(4) | bout(1) | b1(4) | b3(4) | b2(1)
_BOFF = {"bq": 0, "bfc": 3, "bout": 7, "b1": 8, "b3": 12, "b2": 16}
B_COLS = 17


def _build_program():
    from concourse import bacc
    nc = bacc.Bacc(None)

    agent = nc.declare_dram_parameter("agent", [B_LOC, NA, D], F32, isOutput=False)
    lane = nc.declare_dram_parameter("lane", [B_LOC, NL, D], F32, isOutput=False)
    poly = nc.declare_dram_parameter("poly", [B_LOC, NG, D], F32, isOutput=False)
    nm_a = nc.declare_dram_parameter("nm_a", [B_LOC, P, (NA // P) * NA], BF16, isOutput=False)
    nm_l = nc.declare_dram_parameter("nm_l", [B_LOC, P, (NL // P) * NA], BF16, isOutput=False)
    nm_g = nc.declare_dram_parameter("nm_g", [B_LOC, P, (NG // P) * NA], BF16, isOutput=False)
    wpack = nc.declare_dram_parameter("wpack", [P, W_COLS], BF16, isOutput=False)
    bpack = nc.declare_dram_parameter("bpack", [P, B_COLS], F32, isOutput=False)
    out = nc.declare_dram_parameter("out", [B_LOC, NA, D], F32, isOutput=True)

    srcs = {"a2a": agent, "l2a": lane, "g2a": poly}
    nms = {"a2a": nm_a, "l2a": nm_l, "g2a": nm_g}

    with tile.TileContext(nc) as tc:
        from contextlib import ExitStack
        with ExitStack() as ctx:
            cpool = ctx.enter_context(tc.tile_pool(name="const", bufs=1))
            inp = ctx.enter_context(tc.tile_pool(name="inp", bufs=3))
            act = ctx.enter_context(tc.tile_pool(name="act", bufs=3))
            small = ctx.enter_context(tc.tile_pool(name="small", bufs=4))
            ptp = ctx.enter_context(tc.tile_pool(name="ptp", bufs=4))
            core = ctx.enter_context(tc.tile_pool(name="core", bufs=1))
            mm = ctx.enter_context(tc.tile_pool(name="mm", bufs=3, space="PSUM"))
            pop = ctx.enter_context(tc.tile_pool(name="pop", bufs=2, space="PSUM"))
            ptq = ctx.enter_context(tc.tile_pool(name="ptq", bufs=3, space="PSUM"))

            # ---- constants -------------------------------------------------
            w_sb = cpool.tile([P, W_COLS], BF16)
            nc.sync.dma_start(w_sb[:], wpack[:])
            b_sb = cpool.tile([P, B_COLS], F32)
            nc.sync.dma_start(b_sb[:], bpack[:])
            id_bf = cpool.tile([P, P], BF16)
            make_identity(nc, id_bf[:])
            id_f32 = cpool.tile([P, P], F32)
            make_identity(nc, id_f32[:])
            eps_sb = cpool.tile([P, 1], F32)
            nc.vector.memset(eps_sb[:], EPS)
            sim_compat = bool(int(os.environ.get("KERNEL_SIM_COMPAT", "0")))

            def wslice(name, c0, n):
                return w_sb[:, _OFF[name] + c0:_OFF[name] + c0 + n]

            def bcol(name, j=0):
                return b_sb[:, _BOFF[name] + j:_BOFF[name] + j + 1]

            # per-relation, per-core-level activation buffers
            oT_all = {r: core.tile([P, B_LOC * P], BF16, tag=f"oT_{r}",
                                   name=f"oT_{r}")
                      for r, _, _ in RELS}
            agT_all = core.tile([P, B_LOC * P], BF16, tag="agT")
            hT_all = core.tile([P, B_LOC * P], F32, tag="hT")
            hnT_all = core.tile([P, B_LOC * P], BF16, tag="hnT")
            g_all = core.tile([P, 4, B_LOC * P], BF16, tag="g")
            outT_all = core.tile([P, B_LOC * P], F32, tag="outT")

            def ln_stats(mv, st6, x_sb, T, base):
                """bn stats for T tiles of x_sb into mv[:, base:base+T, :]."""
                for t in range(T):
                    nc.vector.bn_stats(st6[:, base + t, :], x_sb[:, t, :])
                    nc.vector.bn_aggr(mv[:, base + t, :], st6[:, base + t, :])

            def ln_rstd(rstd, mv, n):
                """rstd[:, :n] = 1/sqrt(var+eps), one batched ACT pass.

                On HW: exp(-0.5*ln(var+eps)) keeps the ACT engine in the
                natural_log_exp table (no table-load thrash against the
                attention exps).  CoreSim lacks Ln, so a sim-compat build
                uses Sqrt + DVE reciprocal instead."""
                if sim_compat:
                    nc.scalar.activation(rstd[:, :n], mv[:, :n, 1], AF.Sqrt,
                                         bias=eps_sb[:])
                    nc.vector.reciprocal(rstd[:, :n], rstd[:, :n])
                else:
                    nc.scalar.activation(rstd[:, :n], mv[:, :n, 1], AF.Ln,
                                         bias=eps_sb[:])
                    nc.scalar.activation(rstd[:, :n], rstd[:, :n], AF.Exp,
                                         scale=-0.5)

            def ln_apply(xn_dst, x_sb, mv, rstd, T, base):
                # apply on the (otherwise idle) Pool engine: 1-tensor-input op
                for t in range(T):
                    nc.gpsimd.tensor_scalar(
                        xn_dst[:, t, :], x_sb[:, t, :],
                        mv[:, base + t, 0:1], rstd[:, base + t:base + t + 1],
                        ALU.subtract, ALU.mult)

            _evac_flip = [0]

            def transpose_128(dst_sb, src_sb, ident, dtype, evac=None):
                """dst_sb[128,128] = src_sb[128,128].T via PE (through PSUM).

                The PSUM evacuation copy alternates DVE / ACT for balance
                unless an engine is forced via `evac`.
                """
                pt = ptq.tile([P, P], dtype, tag="pt")
                nc.tensor.transpose(pt[:], src_sb[:], ident)
                if evac is None:
                    _evac_flip[0] ^= 1
                    evac = nc.vector if _evac_flip[0] else nc.scalar
                if evac is nc.scalar:
                    nc.scalar.copy(dst_sb, pt[:])
                else:
                    nc.vector.tensor_copy(dst_sb, pt[:])

            N_TILES = sum(T for _, _, T in RELS)   # 13
            for s in range(B_LOC):
                # ---- phase A: LN + projections -----------------------------
                xnT = {}
                v_aug = {}
                kT = {}
                qT = act.tile([P, 3, P], BF16, tag="qT")
                # DMA + LN stats for all sources, one batched rstd pass
                x_sbs = {}
                st6 = small.tile([P, N_TILES, 6], F32, tag="st6")
                mv = small.tile([P, N_TILES, 2], F32, tag="mv")
                rstd = small.tile([P, N_TILES], F32, tag="rstd")
                base = 0
                bases = {}
                for r, NK, T in RELS:
                    x_sb = inp.tile([P, T, P], F32, tag=f"x_{r}")
                    nc.sync.dma_start(
                        x_sb[:], srcs[r][s].rearrange("(t p) d -> p t d", p=P))
                    x_sbs[r] = x_sb
                    ln_stats(mv, st6, x_sb, T, base)
                    bases[r] = base
                    base += T
                ln_rstd(rstd, mv, N_TILES)
                for r, NK, T in RELS:
                    x_sb = x_sbs[r]
                    xn = act.tile([P, T, P], BF16, tag=f"xn_{r}")
                    ln_apply(xn, x_sb, mv, rstd, T, bases[r])
                    # transpose each token tile -> xnT (feature-major)
                    xT = act.tile([P, T, P], BF16, tag=f"xT_{r}")
                    for t in range(T):
                        transpose_128(xT[:, t, :], xn[:, t, :], id_bf[:], BF16)
                    xnT[r] = xT
                    # k^T = Wk.T @ xn^T   (all tiles in <=512 chunks)
                    kr = act.tile([P, T * P], BF16, tag=f"kT_{r}")
                    kvkey = {"a2a": "kv_o", "l2a": "kv_l", "g2a": "kv_g"}[r]
                    wk = wslice(kvkey, 0, P)
                    for c0 in range(0, T * P, 512):
                        cw = min(512, T * P - c0)
                        pk = mm.tile([P, 512], F32, tag="mm")
                        nc.tensor.matmul(
                            pk[:, :cw], wk,
                            xT.rearrange("p t d -> p (t d)")[:, c0:c0 + cw])
                        nc.scalar.copy(kr[:, c0:c0 + cw], pk[:, :cw])
                    kT[r] = kr
                    # v = xn @ Wv, ones column interleaved: v_aug[128,T,4,33]
                    wv = wslice(kvkey, P, P)
                    va = act.tile([P, T, 4, 33], BF16, tag=f"va_{r}")
                    # ones columns for the fused softmax denominator (same-size
                    # bitcast, so the strided AP is fine for memset)
                    nc.gpsimd.memset(va[:, :, :, 32:33], 1.0)
                    for t in range(T):
                        pv = mm.tile([P, 512], F32, tag="mm")
                        nc.tensor.matmul(pv[:, :P], xT[:, t, :], wv)
                        vsrc = pv[:, :P].rearrange("p (h d) -> p h d", h=H)
                        if t % 2 == 0:
                            nc.scalar.copy(va[:, t, :, 0:32], vsrc)
                        else:
                            nc.vector.tensor_copy(va[:, t, :, 0:32], vsrc)
                    v_aug[r] = va
                    if r == "a2a":
                        # q^T (3 relation blocks) from the agent xn^T
                        for j in range(3):
                            pq = mm.tile([P, 512], F32, tag="mm")
                            nc.tensor.matmul(pq[:, :P], wslice("wq", j * P, P),
                                             xT[:, 0, :])
                            nc.vector.tensor_scalar(
                                qT[:, j, :], pq[:, :P], bcol("bq", j), None,
                                ALU.add)
                        # raw agent^T (bf16) for self_fc
                        xraw = act.tile([P, P], BF16, tag="xraw")
                        nc.vector.tensor_copy(xraw[:], x_sb[:, 0, :])
                        transpose_128(agT_all[:, s * P:(s + 1) * P], xraw[:],
                                      id_bf[:], BF16)

                # ---- phase B: attention ------------------------------------
                for ri, (r, NK, T) in enumerate(RELS):
                    nm_sb = act.tile([P, T, P], BF16, tag=f"nm_{r}")
                    nc.sync.dma_start(
                        nm_sb[:], nms[r][s].rearrange("p (t q) -> p t q", t=T))
                    po = pop.tile([P, H * 33], F32, tag="po")
                    kT3 = kT[r].rearrange("p (t d) -> p t d", d=P)
                    for h in range(H):
                        hs = slice(h * DK, (h + 1) * DK)
                        # all score tiles for this head first (keeps the PE in
                        # 32-row tiling mode), then all o-matmuls (full mode)
                        pt = ptp.tile([P, 8, P], BF16, tag="pT", name="pT")
                        for t0 in range(0, T, 4):
                            g = min(4, T - t0)
                            ps = mm.tile([P, 4, P], F32, tag="mm", name="ps")
                            for j in range(g):
                                nc.tensor.matmul(
                                    ps[:, j, :], kT3[hs, t0 + j, :],
                                    qT[hs, ri, :],
                                    tile_position=(h * DK, 0))
                            nc.scalar.activation(pt[:, t0:t0 + g, :],
                                                 ps[:, :g, :], AF.Exp)
                            # mask multiply: mostly DVE (bf16 2x), 1/4 to Pool
                            eng = nc.gpsimd if (t0 // 4 + h) % 2 == 1 else nc.vector
                            eng.tensor_mul(pt[:, t0:t0 + g, :],
                                           pt[:, t0:t0 + g, :],
                                           nm_sb[:, t0:t0 + g, :])
                        for t in range(T):
                            nc.tensor.matmul(
                                po[:, h * 33:(h + 1) * 33],
                                pt[:, t, :],
                                v_aug[r][:, t, h, :],
                                start=(t == 0), stop=(t == T - 1))
                    # normalize + evacuate o, then transpose to oT_all
                    po3 = po.rearrange("p (h c) -> p h c", c=33)
                    rcp = small.tile([P, H, 1], F32, tag="rcp")
                    nc.vector.reciprocal(rcp[:, :, 0], po3[:, :, 32])
                    o_sb = act.tile([P, P], BF16, tag="o_sb")
                    nc.vector.tensor_tensor(
                        o_sb.rearrange("p (h d) -> p h d", h=H),
                        po3[:, :, 0:32],
                        rcp[:].to_broadcast((P, H, DK)),
                        ALU.mult)
                    transpose_128(oT_all[r][:, s * P:(s + 1) * P], o_sb[:],
                                  id_bf[:], BF16)

            # ---- phase C: fc / out_fc / FFN over all scenes ----------------
            QW = B_LOC * P     # 1024 moving width
            fc_in = [agT_all, oT_all["a2a"], oT_all["l2a"], oT_all["g2a"]]
            # cat order in reference: [h_self, h_other, h_l2a, h_g2a]
            hT = [core.tile([P, QW], BF16, tag=f"hT_{j}", name=f"hT_{j}")
                  for j in range(4)]
            for j in range(4):
                for c0 in range(0, QW, 512):
                    pf = mm.tile([P, 512], F32, tag="mm")
                    nc.tensor.matmul(pf[:], wslice("wfc", j * P, P),
                                     fc_in[j][:, c0:c0 + 512])
                    nc.scalar.activation(hT[j][:, c0:c0 + 512], pf[:],
                                         AF.Relu, bias=bcol("bfc", j))
            for c0 in range(0, QW, 512):
                ph = mm.tile([P, 512], F32, tag="mm")
                for j in range(4):
                    nc.tensor.matmul(ph[:], wslice("wout", j * P, P),
                                     hT[j][:, c0:c0 + 512],
                                     start=(j == 0), stop=(j == 3))
                nc.vector.tensor_scalar(hT_all[:, c0:c0 + 512], ph[:],
                                        bcol("bout"), None, ALU.add)
            # FFN layernorm (token-major transpose round trip); stats for all
            # scenes first so the rstd ACT pass is batched once
            h_tok = core.tile([P, B_LOC, P], F32, tag="h_tok")
            st6f = small.tile([P, B_LOC, 6], F32, tag="st6")
            mvf = small.tile([P, B_LOC, 2], F32, tag="mv")
            rstdf = small.tile([P, B_LOC], F32, tag="rstd")
            for s in range(B_LOC):
                transpose_128(h_tok[:, s, :], hT_all[:, s * P:(s + 1) * P],
                              id_f32[:], F32)
                ln_stats(mvf, st6f, h_tok[:, s:s + 1, :], 1, s)
            ln_rstd(rstdf, mvf, B_LOC)
            for s in range(B_LOC):
                hn = act.tile([P, P], BF16, tag="hn_tok")
                ln_apply(hn.rearrange("p (t d) -> p t d", t=1),
                         h_tok[:, s:s + 1, :], mvf, rstdf, 1, s)
                transpose_128(hnT_all[:, s * P:(s + 1) * P], hn[:],
                              id_bf[:], BF16)
            # w1 (silu) * w3 -> g ; w2 @ g + b2 + h -> outT
            for c0 in range(0, QW, 512):
                for c in range(4):
                    p1 = mm.tile([P, 512], F32, tag="mm")
                    nc.tensor.matmul(p1[:], wslice("w1", c * P, P),
                                     hnT_all[:, c0:c0 + 512])
                    # silu(a) = a * sigmoid(a): sigmoid on ACT, multiply fused
                    # into the PSUM evacuation (CoreSim lacks a Silu table)
                    sg = act.tile([P, 512], BF16, tag="sg")
                    nc.scalar.activation(sg[:], p1[:], AF.Sigmoid,
                                         bias=bcol("b1", c))
                    t1 = act.tile([P, 512], BF16, tag="t1")
                    nc.vector.scalar_tensor_tensor(
                        t1[:], p1[:], bcol("b1", c), sg[:], ALU.add, ALU.mult)
                    p3 = mm.tile([P, 512], F32, tag="mm")
                    nc.tensor.matmul(p3[:], wslice("w3", c * P, P),
                                     hnT_all[:, c0:c0 + 512])
                    nc.vector.scalar_tensor_tensor(
                        g_all[:, c, c0:c0 + 512], p3[:], bcol("b3", c), t1[:],
                        ALU.add, ALU.mult)
                pf = mm.tile([P, 512], F32, tag="mm")
                for c in range(4):
                    nc.tensor.matmul(pf[:], wslice("w2", c * P, P),
                                     g_all[:, c, c0:c0 + 512],
                                     start=(c == 0), stop=(c == 3))
                nc.vector.scalar_tensor_tensor(
                    outT_all[:, c0:c0 + 512], pf[:], bcol("b2"),
                    hT_all[:, c0:c0 + 512], ALU.add, ALU.add)
            for s in range(B_LOC):
                o_tok = act.tile([P, P], F32, tag="o_tok")
                transpose_128(o_tok[:], outT_all[:, s * P:(s + 1) * P],
                              id_f32[:], F32)
                nc.sync.dma_start(out[s], o_tok[:])

    # Constrain the act-table chooser so Exp and Ln resolve to the ONE table
    # that holds both (natural_log_exp_and_others): without this, Exp picks
    # exp_and_others and Ln picks natural_log and every LN<->attention
    # transition pays a ~1.3us ACT table load (61 loads -> ~78us/core).
    # Table positions (= act_func_set ids walrus consumes) are unchanged;
    # we only shrink the competing sets the chooser may select.
    import concourse.bacc as bacc_mod
    _orig_tables = bacc_mod.get_activation_tables

    def _constrained(arch):
        t = {k: set(v) for k, v in _orig_tables(arch).items()}
        for name, s in t.items():
            if name != "natural_log_exp_and_others":
                s.discard(AF.Exp)
                s.discard(AF.Ln)
        return t

    bacc_mod.get_activation_tables = _constrained
    try:
        nc.compile()
    finally:
        bacc_mod.get_activation_tables = _orig_tables
    return nc


_PROG = None


def _get_program():
    global _PROG
    if _PROG is None:
        _PROG = _build_program()
    return _PROG


def _prep_host(params):
    """Fold LN affine params / attention scales / v-biases into weights.

    Returns (wpack[128, W_COLS] bf16, bpack[128, B_COLS] f32).
    """
    import math as _m

    def _np(v):
        if isinstance(v, (tuple, list)):
            return tuple(np.asarray(x, np.float32) for x in v)
        return np.asarray(v, np.float32)

    g = {k: _np(v) for k, v in params.items()}

    def fold(ln, w):
        gam, bet = ln
        return gam[:, None] * w, bet @ w

    wq, bq = fold(g["ln_q"], g["wq"])             # [128,384], [384]
    # attention scale folded into each relation's q block
    for j, nk in enumerate([NA, NL, NG]):
        sc = _m.log(nk + 1.0, 32) / _m.sqrt(DK)
        wq[:, j * P:(j + 1) * P] *= sc
        bq[j * P:(j + 1) * P] *= sc
    kv = {}
    vbias = {}
    for r, lnn, wn in [("o", "ln_other", "wkv_other"),
                       ("l", "ln_l2a", "wkv_l2a"),
                       ("g", "ln_g2a", "wkv_g2a")]:
        w, b = fold(g[lnn], g[wn])                # [128,256], [256]
        kv[r] = w
        kv[r + "_kb"] = b[:P]                      # k bias (zero for this model)
        vbias[r] = b[P:]                           # v bias -> folded into fc bias
    # fc weights / biases in device order (self, other, l2a, g2a), with the
    # v-bias fold: relu((o + vb) @ W + b) = relu(o @ W + (vb @ W + b))
    wfc = np.concatenate([g["self_fc"][0], g["fc_other"][0], g["fc_l2a"][0],
                          g["fc_g2a"][0]], axis=1)         # [128, 4*128]
    bfc = np.stack([
        g["self_fc"][1],
        g["fc_other"][1] + vbias["o"] @ g["fc_other"][0],
        g["fc_l2a"][1] + vbias["l"] @ g["fc_l2a"][0],
        g["fc_g2a"][1] + vbias["g"] @ g["fc_g2a"][0]], axis=1)
    # out_fc: cat order [self, other, l2a, g2a] rows of wout
    wout, bout = g["out_fc"]                       # [512,128],[128]
    wout_p = wout.reshape(4, P, P).transpose(1, 0, 2).reshape(P, 4 * P)
    # ffn
    w1, b1 = g["ffn_w1"]
    w2, b2 = g["ffn_w2"]
    w3, b3 = g["ffn_w3"]
    gam, bet = g["ffn_ln"]
    w1f = gam[:, None] * w1
    b1f = bet @ w1 + b1
    w3f = gam[:, None] * w3
    b3f = bet @ w3 + b3
    w2_p = w2.reshape(4, P, P).transpose(1, 0, 2).reshape(P, 4 * P)

    wpack = np.zeros((P, W_COLS), np.float32)
    wpack[:, _OFF["wq"]:_OFF["wq"] + 384] = wq
    for r, key in [("o", "kv_o"), ("l", "kv_l"), ("g", "kv_g")]:
        wpack[:, _OFF[key]:_OFF[key] + 256] = kv[r]
    wpack[:, _OFF["wfc"]:_OFF["wfc"] + 512] = wfc
    wpack[:, _OFF["wout"]:_OFF["wout"] + 512] = wout_p
    wpack[:, _OFF["w1"]:_OFF["w1"] + 512] = w1f
    wpack[:, _OFF["w3"]:_OFF["w3"] + 512] = w3f
    wpack[:, _OFF["w2"]:_OFF["w2"] + 512] = w2_p

    bpack = np.zeros((P, B_COLS), np.float32)
    bpack[:, _BOFF["bq"]:_BOFF["bq"] + 3] = bq.reshape(3, P).T
    bpack[:, _BOFF["bfc"]:_BOFF["bfc"] + 4] = bfc
    bpack[:, _BOFF["bout"]] = bout
    bpack[:, _BOFF["b1"]:_BOFF["b1"] + 4] = b1f.reshape(4, P).T
    bpack[:, _BOFF["b3"]:_BOFF["b3"] + 4] = b3f.reshape(4, P).T
    bpack[:, _BOFF["b2"]] = b2

    import ml_dtypes
    return wpack.astype(ml_dtypes.bfloat16), bpack


def _prep_mask(mask):
    """bool [Bl, NA, NK] -> notmask^T bf16 [Bl, 128, T*NA] (k-partition-major)."""
    import ml_dtypes
    bl, na, nk = mask.shape
    t = nk // P
    nmT = (~np.asarray(mask, bool)).astype(np.float32).transpose(0, 2, 1)
    nmT = nmT.reshape(bl, t, P, na).transpose(0, 2, 1, 3).reshape(bl, P, t * na)
    return np.ascontiguousarray(nmT.astype(ml_dtypes.bfloat16))


def kernel(agent_h, lane_h, poly_h, a2a_mask, l2a_mask, g2a_mask, params):
    agent_h = np.ascontiguousarray(np.asarray(agent_h, np.float32))
    lane_h = np.ascontiguousarray(np.asarray(lane_h, np.float32))
    poly_h = np.ascontiguousarray(np.asarray(poly_h, np.float32))
    wpack, bpack = _prep_host(params)
    nm_a = _prep_mask(np.asarray(a2a_mask))
    nm_l = _prep_mask(np.asarray(l2a_mask))
    nm_g = _prep_mask(np.asarray(g2a_mask))

    nc = _get_program()
    in_maps = []
    for c in range(N_CORES):
        sl = slice(c * B_LOC, (c + 1) * B_LOC)
        in_maps.append({
            "agent": agent_h[sl], "lane": lane_h[sl], "poly": poly_h[sl],
            "nm_a": nm_a[sl], "nm_l": nm_l[sl], "nm_g": nm_g[sl],
            "wpack": wpack, "bpack": bpack,
        })
    trace = bool(int(os.environ.get("KERNEL_TRACE", "0")))
    res = run_bass_kernel_spmd(nc, in_maps, list(range(N_CORES)), trace=trace)
    if trace and res.exec_time_ns is not None:
        print(f"HW exec time: {res.exec_time_ns} ns")
    out = np.concatenate([res.results[c]["out"] for c in range(N_CORES)], axis=0)
    return out.astype(np.float32)


# revision 44
# speedup vs baseline: 12540.6365x; 1.0100x over previous
"""AgentHetGNN layer on 8 Trainium2 NeuronCores (Bass/Tile, SPMD data-parallel).

Strategy
--------
Data-parallel over the 64 scenes: 8 scenes per core.  Inside a core,
per scene:
  A) LayerNorm (stats on DVE, apply fused sub+mul) + projections.
     Activations move feature-major ("^T", [feat, token]) so matmul
     contractions land on partitions; per-128-token tiles are PE-transposed.
     LN gammas are folded into the projection weights on the host, the
     attention scale is folded into wq, v-projection biases are folded
     into the post-attention fc biases (all exact algebra).
  B) Attention per (relation, head, key-tile): scores are produced
     TRANSPOSED (s^T = k_h^T.T @ q_h^T, K=32 contraction), exp on ACT
     (PSUM->SBUF, bf16), multiplicative mask (notmask, bf16, DVE 4x mode),
     then o = p^T.T @ [v | 1] with a ones-column interleaved into v so the
     softmax denominator accumulates for free in the same PSUM tile.
     No max-subtraction is needed: scores are O(1) by construction and
     masked entries are exactly zeroed after exp.
  C) fc/out_fc/FFN batched across all 8 scenes per core (moving operand
     width 512-1024), silu on ACT, fused (psum+bias)*t / (psum+bias)+h
     on DVE via scalar_tensor_tensor.

All heavy matmuls run in bf16 (1 cyc/row on PE); f32 is kept for inputs,
LN statistics, PSUM accumulation, the residual h, and the final output.
"""

import os
import numpy as np

import concourse.bass as bass
import concourse.mybir as mybir
import concourse.tile as tile
from concourse.bass_utils import run_bass_kernel_spmd
from concourse.masks import make_identity

F32 = mybir.dt.float32
BF16 = mybir.dt.bfloat16
AF = mybir.ActivationFunctionType
ALU = mybir.AluOpType

N_CORES = 8
B = 64
B_LOC = B // N_CORES          # scenes per core
NA, NL, NG, D = 128, 1024, 512, 128
H, DK = 4, 32
P = 128
EPS = 1e-5

# relations: (name, n_key_tokens, tiles-of-128)
RELS = [("a2a", NA, NA // P), ("l2a", NL, NL // P), ("g2a", NG, NG // P)]

# ---------------------------------------------------------------------------
# weight packing offsets (columns in the single packed bf16 weight tensor)
# layout: wq(384) | k_o(128) v_o(128) | k_l(128) v_l(128) | k_g(128) v_g(128)
#         | wfc(4*128) | wout(4*128) | w1(512) | w3(512) | w2(4*128)
_OFF = {}
_c = 0
for _name, _w in [("wq", 384), ("kv_o", 256), ("kv_l", 256), ("kv_g", 256),
                  ("wfc", 512), ("wout", 512), ("w1", 512), ("w3", 512),
                  ("w2", 512)]:
    _OFF[_name] = _c
    _c += _w
W_COLS = _c

# bias packing: bq(3) | bfc(4) | bout(1) | b1(4) | b3(4) | b2(1)
_BOFF = {"bq": 0, "bfc": 3, "bout": 7, "b1": 8, "b3": 12, "b2": 16}
B_COLS = 17


def _build_program():
    from concourse import bacc
    nc = bacc.Bacc(None)

    agent = nc.declare_dram_parameter("agent", [B_LOC, NA, D], F32, isOutput=False)
    lane = nc.declare_dram_parameter("lane", [B_LOC, NL, D], F32, isOutput=False)
    poly = nc.declare_dram_parameter("poly", [B_LOC, NG, D], F32, isOutput=False)
    nm_a = nc.declare_dram_parameter("nm_a", [B_LOC, P, (NA // P) * NA], BF16, isOutput=False)
    nm_l = nc.declare_dram_parameter("nm_l", [B_LOC, P, (NL // P) * NA], BF16, isOutput=False)
    nm_g = nc.declare_dram_parameter("nm_g", [B_LOC, P, (NG // P) * NA], BF16, isOutput=False)
    wpack = nc.declare_dram_parameter("wpack", [P, W_COLS], BF16, isOutput=False)
    bpack = nc.declare_dram_parameter("bpack", [P, B_COLS], F32, isOutput=False)
    out = nc.declare_dram_parameter("out", [B_LOC, NA, D], F32, isOutput=True)

    srcs = {"a2a": agent, "l2a": lane, "g2a": poly}
    nms = {"a2a": nm_a, "l2a": nm_l, "g2a": nm_g}

    with tile.TileContext(nc) as tc:
        from contextlib import ExitStack
        with ExitStack() as ctx:
            cpool = ctx.enter_context(tc.tile_pool(name="const", bufs=1))
            inp = ctx.enter_context(tc.tile_pool(name="inp", bufs=4))
            act = ctx.enter_context(tc.tile_pool(name="act", bufs=4))
            small = ctx.enter_context(tc.tile_pool(name="small", bufs=6))
            ptp = ctx.enter_context(tc.tile_pool(name="ptp", bufs=6))
            core = ctx.enter_context(tc.tile_pool(name="core", bufs=1))
            mm = ctx.enter_context(tc.tile_pool(name="mm", bufs=3, space="PSUM"))
            pop = ctx.enter_context(tc.tile_pool(name="pop", bufs=2, space="PSUM"))
            ptq = ctx.enter_context(tc.tile_pool(name="ptq", bufs=3, space="PSUM"))

            # ---- constants -------------------------------------------------
            w_sb = cpool.tile([P, W_COLS], BF16)
            nc.sync.dma_start(w_sb[:], wpack[:])
            b_sb = cpool.tile([P, B_COLS], F32)
            nc.sync.dma_start(b_sb[:], bpack[:])
            id_bf = cpool.tile([P, P], BF16)
            make_identity(nc, id_bf[:])
            id_f32 = cpool.tile([P, P], F32)
            make_identity(nc, id_f32[:])
            eps_sb = cpool.tile([P, 1], F32)
            nc.vector.memset(eps_sb[:], EPS)
            sim_compat = bool(int(os.environ.get("KERNEL_SIM_COMPAT", "0")))

            def wslice(name, c0, n):
                return w_sb[:, _OFF[name] + c0:_OFF[name] + c0 + n]

            def bcol(name, j=0):
                return b_sb[:, _BOFF[name] + j:_BOFF[name] + j + 1]

            # per-relation, per-core-level activation buffers
            oT_all = {r: core.tile([P, B_LOC * P], BF16, tag=f"oT_{r}",
                                   name=f"oT_{r}")
                      for r, _, _ in RELS}
            agT_all = core.tile([P, B_LOC * P], BF16, tag="agT")
            hT_all = core.tile([P, B_LOC * P], F32, tag="hT")
            hnT_all = core.tile([P, B_LOC * P], BF16, tag="hnT")
            g_all = core.tile([P, 4, B_LOC * P], BF16, tag="g")
            outT_all = core.tile([P, B_LOC * P], F32, tag="outT")

            def ln_stats(mv, st6, x_sb, T, base):
                """bn stats for T tiles of x_sb into mv[:, base:base+T, :]."""
                for t in range(T):
                    nc.vector.bn_stats(st6[:, base + t, :], x_sb[:, t, :])
                    nc.vector.bn_aggr(mv[:, base + t, :], st6[:, base + t, :])

            def ln_rstd(rstd, mv, n):
                """rstd[:, :n] = 1/sqrt(var+eps), one batched ACT pass.

                On HW: exp(-0.5*ln(var+eps)) keeps the ACT engine in the
                natural_log_exp table (no table-load thrash against the
                attention exps).  CoreSim lacks Ln, so a sim-compat build
                uses Sqrt + DVE reciprocal instead."""
                if sim_compat:
                    nc.scalar.activation(rstd[:, :n], mv[:, :n, 1], AF.Sqrt,
                                         bias=eps_sb[:])
                    nc.vector.reciprocal(rstd[:, :n], rstd[:, :n])
                else:
                    nc.scalar.activation(rstd[:, :n], mv[:, :n, 1], AF.Ln,
                                         bias=eps_sb[:])
                    nc.scalar.activation(rstd[:, :n], rstd[:, :n], AF.Exp,
                                         scale=-0.5)

            def ln_apply(xn_dst, x_sb, mv, rstd, T, base):
                # apply on the (otherwise idle) Pool engine: 1-tensor-input op
                for t in range(T):
                    nc.gpsimd.tensor_scalar(
                        xn_dst[:, t, :], x_sb[:, t, :],
                        mv[:, base + t, 0:1], rstd[:, base + t:base + t + 1],
                        ALU.subtract, ALU.mult)

            _evac_flip = [0]

            def transpose_128(dst_sb, src_sb, ident, dtype, evac=None):
                """dst_sb[128,128] = src_sb[128,128].T via PE (through PSUM).

                The PSUM evacuation copy alternates DVE / ACT for balance
                unless an engine is forced via `evac`.
                """
                pt = ptq.tile([P, P], dtype, tag="pt")
                nc.tensor.transpose(pt[:], src_sb[:], ident)
                if evac is None:
                    _evac_flip[0] ^= 1
                    evac = nc.vector if _evac_flip[0] else nc.scalar
                if evac is nc.scalar:
                    nc.scalar.copy(dst_sb, pt[:])
                else:
                    nc.vector.tensor_copy(dst_sb, pt[:])

            N_TILES = sum(T for _, _, T in RELS)   # 13
            for s in range(B_LOC):
                # ---- phase A: LN + projections -----------------------------
                xnT = {}
                v_aug = {}
                kT = {}
                qT = act.tile([P, 3, P], BF16, tag="qT")
                # DMA + LN stats for all sources, one batched rstd pass
                x_sbs = {}
                st6 = small.tile([P, N_TILES, 6], F32, tag="st6")
                mv = small.tile([P, N_TILES, 2], F32, tag="mv")
                rstd = small.tile([P, N_TILES], F32, tag="rstd")
                base = 0
                bases = {}
                for r, NK, T in RELS:
                    x_sb = inp.tile([P, T, P], F32, tag=f"x_{r}")
                    nc.sync.dma_start(
                        x_sb[:], srcs[r][s].rearrange("(t p) d -> p t d", p=P))
                    x_sbs[r] = x_sb
                    ln_stats(mv, st6, x_sb, T, base)
                    bases[r] = base
                    base += T
                ln_rstd(rstd, mv, N_TILES)
                for r, NK, T in RELS:
                    x_sb = x_sbs[r]
                    xn = act.tile([P, T, P], BF16, tag=f"xn_{r}")
                    ln_apply(xn, x_sb, mv, rstd, T, bases[r])
                    # transpose each token tile -> xnT (feature-major)
                    xT = act.tile([P, T, P], BF16, tag=f"xT_{r}")
                    for t in range(T):
                        transpose_128(xT[:, t, :], xn[:, t, :], id_bf[:], BF16)
                    xnT[r] = xT
                    # k^T = Wk.T @ xn^T   (all tiles in <=512 chunks)
                    kr = act.tile([P, T * P], BF16, tag=f"kT_{r}")
                    kvkey = {"a2a": "kv_o", "l2a": "kv_l", "g2a": "kv_g"}[r]
                    wk = wslice(kvkey, 0, P)
                    for c0 in range(0, T * P, 512):
                        cw = min(512, T * P - c0)
                        pk = mm.tile([P, 512], F32, tag="mm")
                        nc.tensor.matmul(
                            pk[:, :cw], wk,
                            xT.rearrange("p t d -> p (t d)")[:, c0:c0 + cw])
                        if (c0 // 512) % 2 == 0:
                            nc.vector.tensor_copy(kr[:, c0:c0 + cw], pk[:, :cw])
                        else:
                            nc.scalar.copy(kr[:, c0:c0 + cw], pk[:, :cw])
                    kT[r] = kr
                    # v = xn @ Wv, ones column interleaved: v_aug[128,T,4,33]
                    wv = wslice(kvkey, P, P)
                    va = act.tile([P, T, 4, 33], BF16, tag=f"va_{r}")
                    # ones columns for the fused softmax denominator (same-size
                    # bitcast, so the strided AP is fine for memset)
                    nc.gpsimd.memset(va[:, :, :, 32:33], 1.0)
                    for t in range(T):
                        pv = mm.tile([P, 512], F32, tag="mm")
                        nc.tensor.matmul(pv[:, :P], xT[:, t, :], wv)
                        vsrc = pv[:, :P].rearrange("p (h d) -> p h d", h=H)
                        if t % 2 == 0:
                            nc.scalar.copy(va[:, t, :, 0:32], vsrc)
                        else:
                            nc.vector.tensor_copy(va[:, t, :, 0:32], vsrc)
                    v_aug[r] = va
                    if r == "a2a":
                        # q^T (3 relation blocks) from the agent xn^T
                        for j in range(3):
                            pq = mm.tile([P, 512], F32, tag="mm")
                            nc.tensor.matmul(pq[:, :P], wslice("wq", j * P, P),
                                             xT[:, 0, :])
                            nc.vector.tensor_scalar(
                                qT[:, j, :], pq[:, :P], bcol("bq", j), None,
                                ALU.add)
                        # raw agent^T (bf16) for self_fc
                        xraw = act.tile([P, P], BF16, tag="xraw")
                        nc.vector.tensor_copy(xraw[:], x_sb[:, 0, :])
                        transpose_128(agT_all[:, s * P:(s + 1) * P], xraw[:],
                                      id_bf[:], BF16)

                # ---- phase B: attention ------------------------------------
                for ri, (r, NK, T) in enumerate(RELS):
                    nm_sb = act.tile([P, T, P], BF16, tag=f"nm_{r}")
                    nc.sync.dma_start(
                        nm_sb[:], nms[r][s].rearrange("p (t q) -> p t q", t=T))
                    po = pop.tile([P, H * 33], F32, tag="po")
                    kT3 = kT[r].rearrange("p (t d) -> p t d", d=P)
                    for h in range(H):
                        hs = slice(h * DK, (h + 1) * DK)
                        # all score tiles for this head first (keeps the PE in
                        # 32-row tiling mode), then all o-matmuls (full mode)
                        pt = ptp.tile([P, 8, P], BF16, tag="pT", name="pT")
                        for t0 in range(0, T, 4):
                            g = min(4, T - t0)
                            ps = mm.tile([P, 4, P], F32, tag="mm", name="ps")
                            for j in range(g):
                                nc.tensor.matmul(
                                    ps[:, j, :], kT3[hs, t0 + j, :],
                                    qT[hs, ri, :],
                                    tile_position=(h * DK, 0))
                            nc.scalar.activation(pt[:, t0:t0 + g, :],
                                                 ps[:, :g, :], AF.Exp)
                            # mask multiply: mostly DVE (bf16 2x), 1/4 to Pool
                            eng = nc.gpsimd if (t0 // 4 + h) % 2 == 1 else nc.vector
                            eng.tensor_mul(pt[:, t0:t0 + g, :],
                                           pt[:, t0:t0 + g, :],
                                           nm_sb[:, t0:t0 + g, :])
                        for t in range(T):
                            nc.tensor.matmul(
                                po[:, h * 33:(h + 1) * 33],
                                pt[:, t, :],
                                v_aug[r][:, t, h, :],
                                start=(t == 0), stop=(t == T - 1))
                    # normalize + evacuate o, then transpose to oT_all
                    po3 = po.rearrange("p (h c) -> p h c", c=33)
                    rcp = small.tile([P, H, 1], F32, tag="rcp")
                    nc.vector.reciprocal(rcp[:, :, 0], po3[:, :, 32])
                    o_sb = act.tile([P, P], BF16, tag="o_sb")
                    nc.vector.tensor_tensor(
                        o_sb.rearrange("p (h d) -> p h d", h=H),
                        po3[:, :, 0:32],
                        rcp[:].to_broadcast((P, H, DK)),
                        ALU.mult)
                    transpose_128(oT_all[r][:, s * P:(s + 1) * P], o_sb[:],
                                  id_bf[:], BF16)

            # ---- phase C: fc / out_fc / FFN over all scenes ----------------
            QW = B_LOC * P     # 1024 moving width
            fc_in = [agT_all, oT_all["a2a"], oT_all["l2a"], oT_all["g2a"]]
            # cat order in reference: [h_self, h_other, h_l2a, h_g2a]
            hT = [core.tile([P, QW], BF16, tag=f"hT_{j}", name=f"hT_{j}")
                  for j in range(4)]
            for j in range(4):
                for c0 in range(0, QW, 512):
                    pf = mm.tile([P, 512], F32, tag="mm")
                    nc.tensor.matmul(pf[:], wslice("wfc", j * P, P),
                                     fc_in[j][:, c0:c0 + 512])
                    nc.scalar.activation(hT[j][:, c0:c0 + 512], pf[:],
                                         AF.Relu, bias=bcol("bfc", j))
            for c0 in range(0, QW, 512):
                ph = mm.tile([P, 512], F32, tag="mm")
                for j in range(4):
                    nc.tensor.matmul(ph[:], wslice("wout", j * P, P),
                                     hT[j][:, c0:c0 + 512],
                                     start=(j == 0), stop=(j == 3))
                nc.vector.tensor_scalar(hT_all[:, c0:c0 + 512], ph[:],
                                        bcol("bout"), None, ALU.add)
            # FFN layernorm (token-major transpose round trip); stats for all
            # scenes first so the rstd ACT pass is batched once
            h_tok = core.tile([P, B_LOC, P], F32, tag="h_tok")
            st6f = small.tile([P, B_LOC, 6], F32, tag="st6")
            mvf = small.tile([P, B_LOC, 2], F32, tag="mv")
            rstdf = small.tile([P, B_LOC], F32, tag="rstd")
            for s in range(B_LOC):
                transpose_128(h_tok[:, s, :], hT_all[:, s * P:(s + 1) * P],
                              id_f32[:], F32)
                ln_stats(mvf, st6f, h_tok[:, s:s + 1, :], 1, s)
            ln_rstd(rstdf, mvf, B_LOC)
            for s in range(B_LOC):
                hn = act.tile([P, P], BF16, tag="hn_tok")
                ln_apply(hn.rearrange("p (t d) -> p t d", t=1),
                         h_tok[:, s:s + 1, :], mvf, rstdf, 1, s)
                transpose_128(hnT_all[:, s * P:(s + 1) * P], hn[:],
                              id_bf[:], BF16)
            # w1 (silu) * w3 -> g ; w2 @ g + b2 + h -> outT
            for c0 in range(0, QW, 512):
                for c in range(4):
                    p1 = mm.tile([P, 512], F32, tag="mm")
                    nc.tensor.matmul(p1[:], wslice("w1", c * P, P),
                                     hnT_all[:, c0:c0 + 512])
                    # silu(a) = a * sigmoid(a): sigmoid on ACT, multiply fused
                    # into the PSUM evacuation (CoreSim lacks a Silu table)
                    sg = act.tile([P, 512], BF16, tag="sg")
                    nc.scalar.activation(sg[:], p1[:], AF.Sigmoid,
                                         bias=bcol("b1", c))
                    t1 = act.tile([P, 512], BF16, tag="t1")
                    nc.vector.scalar_tensor_tensor(
                        t1[:], p1[:], bcol("b1", c), sg[:], ALU.add, ALU.mult)
                    p3 = mm.tile([P, 512], F32, tag="mm")
                    nc.tensor.matmul(p3[:], wslice("w3", c * P, P),
                                     hnT_all[:, c0:c0 + 512])
                    nc.vector.scalar_tensor_tensor(
                        g_all[:, c, c0:c0 + 512], p3[:], bcol("b3", c), t1[:],
                        ALU.add, ALU.mult)
                pf = mm.tile([P, 512], F32, tag="mm")
                for c in range(4):
                    nc.tensor.matmul(pf[:], wslice("w2", c * P, P),
                                     g_all[:, c, c0:c0 + 512],
                                     start=(c == 0), stop=(c == 3))
                nc.vector.scalar_tensor_tensor(
                    outT_all[:, c0:c0 + 512], pf[:], bcol("b2"),
                    hT_all[:, c0:c0 + 512], ALU.add, ALU.add)
            for s in range(B_LOC):
                o_tok = act.tile([P, P], F32, tag="o_tok")
                transpose_128(o_tok[:], outT_all[:, s * P:(s + 1) * P],
                              id_f32[:], F32)
                nc.sync.dma_start(out[s], o_tok[:])

    # Constrain the act-table chooser so Exp and Ln resolve to the ONE table
    # that holds both (natural_log_exp_and_others): without this, Exp picks
    # exp_and_others and Ln picks natural_log and every LN<->attention
    # transition pays a ~1.3us ACT table load (61 loads -> ~78us/core).
    # Table positions (= act_func_set ids walrus consumes) are unchanged;
    # we only shrink the competing sets the chooser may select.
    import concourse.bacc as bacc_mod
    _orig_tables = bacc_mod.get_activation_tables

    def _constrained(arch):
        t = {k: set(v) for k, v in _orig_tables(arch).items()}
        for name, s in t.items():
            if name != "natural_log_exp_and_others":
                s.discard(AF.Exp)
                s.discard(AF.Ln)
        return t

    bacc_mod.get_activation_tables = _constrained
    try:
        nc.compile()
    finally:
        bacc_mod.get_activation_tables = _orig_tables
    return nc


_PROG = None


def _get_program():
    global _PROG
    if _PROG is None:
        _PROG = _build_program()
    return _PROG


def _prep_host(params):
    """Fold LN affine params / attention scales / v-biases into weights.

    Returns (wpack[128, W_COLS] bf16, bpack[128, B_COLS] f32).
    """
    import math as _m

    def _np(v):
        if isinstance(v, (tuple, list)):
            return tuple(np.asarray(x, np.float32) for x in v)
        return np.asarray(v, np.float32)

    g = {k: _np(v) for k, v in params.items()}

    def fold(ln, w):
        gam, bet = ln
        return gam[:, None] * w, bet @ w

    wq, bq = fold(g["ln_q"], g["wq"])             # [128,384], [384]
    # attention scale folded into each relation's q block
    for j, nk in enumerate([NA, NL, NG]):
        sc = _m.log(nk + 1.0, 32) / _m.sqrt(DK)
        wq[:, j * P:(j + 1) * P] *= sc
        bq[j * P:(j + 1) * P] *= sc
    kv = {}
    vbias = {}
    for r, lnn, wn in [("o", "ln_other", "wkv_other"),
                       ("l", "ln_l2a", "wkv_l2a"),
                       ("g", "ln_g2a", "wkv_g2a")]:
        w, b = fold(g[lnn], g[wn])                # [128,256], [256]
        kv[r] = w
        kv[r + "_kb"] = b[:P]                      # k bias (zero for this model)
        vbias[r] = b[P:]                           # v bias -> folded into fc bias
    # fc weights / biases in device order (self, other, l2a, g2a), with the
    # v-bias fold: relu((o + vb) @ W + b) = relu(o @ W + (vb @ W + b))
    wfc = np.concatenate([g["self_fc"][0], g["fc_other"][0], g["fc_l2a"][0],
                          g["fc_g2a"][0]], axis=1)         # [128, 4*128]
    bfc = np.stack([
        g["self_fc"][1],
        g["fc_other"][1] + vbias["o"] @ g["fc_other"][0],
        g["fc_l2a"][1] + vbias["l"] @ g["fc_l2a"][0],
        g["fc_g2a"][1] + vbias["g"] @ g["fc_g2a"][0]], axis=1)
    # out_fc: cat order [self, other, l2a, g2a] rows of wout
    wout, bout = g["out_fc"]                       # [512,128],[128]
    wout_p = wout.reshape(4, P, P).transpose(1, 0, 2).reshape(P, 4 * P)
    # ffn
    w1, b1 = g["ffn_w1"]
    w2, b2 = g["ffn_w2"]
    w3, b3 = g["ffn_w3"]
    gam, bet = g["ffn_ln"]
    w1f = gam[:, None] * w1
    b1f = bet @ w1 + b1
    w3f = gam[:, None] * w3
    b3f = bet @ w3 + b3
    w2_p = w2.reshape(4, P, P).transpose(1, 0, 2).reshape(P, 4 * P)

    wpack = np.zeros((P, W_COLS), np.float32)
    wpack[:, _OFF["wq"]:_OFF["wq"] + 384] = wq
    for r, key in [("o", "kv_o"), ("l", "kv_l"), ("g", "kv_g")]:
        wpack[:, _OFF[key]:_OFF[key] + 256] = kv[r]
    wpack[:, _OFF["wfc"]:_OFF["wfc"] + 512] = wfc
    wpack[:, _OFF["wout"]:_OFF["wout"] + 512] = wout_p
    wpack[:, _OFF["w1"]:_OFF["w1"] + 512] = w1f
    wpack[:, _OFF["w3"]:_OFF["w3"] + 512] = w3f
    wpack[:, _OFF["w2"]:_OFF["w2"] + 512] = w2_p

    bpack = np.zeros((P, B_COLS), np.float32)
    bpack[:, _BOFF["bq"]:_BOFF["bq"] + 3] = bq.reshape(3, P).T
    bpack[:, _BOFF["bfc"]:_BOFF["bfc"] + 4] = bfc
    bpack[:, _BOFF["bout"]] = bout
    bpack[:, _BOFF["b1"]:_BOFF["b1"] + 4] = b1f.reshape(4, P).T
    bpack[:, _BOFF["b3"]:_BOFF["b3"] + 4] = b3f.reshape(4, P).T
    bpack[:, _BOFF["b2"]] = b2

    import ml_dtypes
    return wpack.astype(ml_dtypes.bfloat16), bpack


def _prep_mask(mask):
    """bool [Bl, NA, NK] -> notmask^T bf16 [Bl, 128, T*NA] (k-partition-major)."""
    import ml_dtypes
    bl, na, nk = mask.shape
    t = nk // P
    nmT = (~np.asarray(mask, bool)).astype(np.float32).transpose(0, 2, 1)
    nmT = nmT.reshape(bl, t, P, na).transpose(0, 2, 1, 3).reshape(bl, P, t * na)
    return np.ascontiguousarray(nmT.astype(ml_dtypes.bfloat16))


def kernel(agent_h, lane_h, poly_h, a2a_mask, l2a_mask, g2a_mask, params):
    agent_h = np.ascontiguousarray(np.asarray(agent_h, np.float32))
    lane_h = np.ascontiguousarray(np.asarray(lane_h, np.float32))
    poly_h = np.ascontiguousarray(np.asarray(poly_h, np.float32))
    wpack, bpack = _prep_host(params)
    nm_a = _prep_mask(np.asarray(a2a_mask))
    nm_l = _prep_mask(np.asarray(l2a_mask))
    nm_g = _prep_mask(np.asarray(g2a_mask))

    nc = _get_program()
    in_maps = []
    for c in range(N_CORES):
        sl = slice(c * B_LOC, (c + 1) * B_LOC)
        in_maps.append({
            "agent": agent_h[sl], "lane": lane_h[sl], "poly": poly_h[sl],
            "nm_a": nm_a[sl], "nm_l": nm_l[sl], "nm_g": nm_g[sl],
            "wpack": wpack, "bpack": bpack,
        })
    trace = bool(int(os.environ.get("KERNEL_TRACE", "0")))
    res = run_bass_kernel_spmd(nc, in_maps, list(range(N_CORES)), trace=trace)
    if trace and res.exec_time_ns is not None:
        print(f"HW exec time: {res.exec_time_ns} ns")
    out = np.concatenate([res.results[c]["out"] for c in range(N_CORES)], axis=0)
    return out.astype(np.float32)


# revision 45
# speedup vs baseline: 12591.0304x; 1.0040x over previous
"""AgentHetGNN layer on 8 Trainium2 NeuronCores (Bass/Tile, SPMD data-parallel).

Strategy
--------
Data-parallel over the 64 scenes: 8 scenes per core.  Inside a core,
per scene:
  A) LayerNorm (stats on DVE, apply fused sub+mul) + projections.
     Activations move feature-major ("^T", [feat, token]) so matmul
     contractions land on partitions; per-128-token tiles are PE-transposed.
     LN gammas are folded into the projection weights on the host, the
     attention scale is folded into wq, v-projection biases are folded
     into the post-attention fc biases (all exact algebra).
  B) Attention per (relation, head, key-tile): scores are produced
     TRANSPOSED (s^T = k_h^T.T @ q_h^T, K=32 contraction), exp on ACT
     (PSUM->SBUF, bf16), multiplicative mask (notmask, bf16, DVE 4x mode),
     then o = p^T.T @ [v | 1] with a ones-column interleaved into v so the
     softmax denominator accumulates for free in the same PSUM tile.
     No max-subtraction is needed: scores are O(1) by construction and
     masked entries are exactly zeroed after exp.
  C) fc/out_fc/FFN batched across all 8 scenes per core (moving operand
     width 512-1024), silu on ACT, fused (psum+bias)*t / (psum+bias)+h
     on DVE via scalar_tensor_tensor.

All heavy matmuls run in bf16 (1 cyc/row on PE); f32 is kept for inputs,
LN statistics, PSUM accumulation, the residual h, and the final output.
"""

import os
import numpy as np

import concourse.bass as bass
import concourse.mybir as mybir
import concourse.tile as tile
from concourse.bass_utils import run_bass_kernel_spmd
from concourse.masks import make_identity

F32 = mybir.dt.float32
BF16 = mybir.dt.bfloat16
AF = mybir.ActivationFunctionType
ALU = mybir.AluOpType

N_CORES = 8
B = 64
B_LOC = B // N_CORES          # scenes per core
NA, NL, NG, D = 128, 1024, 512, 128
H, DK = 4, 32
P = 128
EPS = 1e-5

# relations: (name, n_key_tokens, tiles-of-128)
RELS = [("a2a", NA, NA // P), ("l2a", NL, NL // P), ("g2a", NG, NG // P)]

# ---------------------------------------------------------------------------
# weight packing offsets (columns in the single packed bf16 weight tensor)
# layout: wq(384) | k_o(128) v_o(128) | k_l(128) v_l(128) | k_g(128) v_g(128)
#         | wfc(4*128) | wout(4*128) | w1(512) | w3(512) | w2(4*128)
_OFF = {}
_c = 0
for _name, _w in [("wq", 384), ("kv_o", 256), ("kv_l", 256), ("kv_g", 256),
                  ("wfc", 512), ("wout", 512), ("w1", 512), ("w3", 512),
                  ("w2", 512)]:
    _OFF[_name] = _c
    _c += _w
W_COLS = _c

# bias packing: bq(3) | bfc(4) | bout(1) | b1(4) | b3(4) | b2(1)
_BOFF = {"bq": 0, "bfc": 3, "bout": 7, "b1": 8, "b3": 12, "b2": 16}
B_COLS = 17


def _build_program():
    from concourse import bacc
    nc = bacc.Bacc(None)

    agent = nc.declare_dram_parameter("agent", [B_LOC, NA, D], F32, isOutput=False)
    lane = nc.declare_dram_parameter("lane", [B_LOC, NL, D], F32, isOutput=False)
    poly = nc.declare_dram_parameter("poly", [B_LOC, NG, D], F32, isOutput=False)
    nm_a = nc.declare_dram_parameter("nm_a", [B_LOC, P, (NA // P) * NA], BF16, isOutput=False)
    nm_l = nc.declare_dram_parameter("nm_l", [B_LOC, P, (NL // P) * NA], BF16, isOutput=False)
    nm_g = nc.declare_dram_parameter("nm_g", [B_LOC, P, (NG // P) * NA], BF16, isOutput=False)
    wpack = nc.declare_dram_parameter("wpack", [P, W_COLS], BF16, isOutput=False)
    bpack = nc.declare_dram_parameter("bpack", [P, B_COLS], F32, isOutput=False)
    out = nc.declare_dram_parameter("out", [B_LOC, NA, D], F32, isOutput=True)

    srcs = {"a2a": agent, "l2a": lane, "g2a": poly}
    nms = {"a2a": nm_a, "l2a": nm_l, "g2a": nm_g}

    with tile.TileContext(nc) as tc:
        from contextlib import ExitStack
        with ExitStack() as ctx:
            cpool = ctx.enter_context(tc.tile_pool(name="const", bufs=1))
            inp = ctx.enter_context(tc.tile_pool(name="inp", bufs=4))
            act = ctx.enter_context(tc.tile_pool(name="act", bufs=4))
            small = ctx.enter_context(tc.tile_pool(name="small", bufs=6))
            ptp = ctx.enter_context(tc.tile_pool(name="ptp", bufs=6))
            core = ctx.enter_context(tc.tile_pool(name="core", bufs=1))
            mm = ctx.enter_context(tc.tile_pool(name="mm", bufs=3, space="PSUM"))
            pop = ctx.enter_context(tc.tile_pool(name="pop", bufs=2, space="PSUM"))
            ptq = ctx.enter_context(tc.tile_pool(name="ptq", bufs=3, space="PSUM"))

            # ---- constants -------------------------------------------------
            w_sb = cpool.tile([P, W_COLS], BF16)
            nc.sync.dma_start(w_sb[:], wpack[:])
            b_sb = cpool.tile([P, B_COLS], F32)
            nc.sync.dma_start(b_sb[:], bpack[:])
            id_bf = cpool.tile([P, P], BF16)
            make_identity(nc, id_bf[:])
            id_f32 = cpool.tile([P, P], F32)
            make_identity(nc, id_f32[:])
            eps_sb = cpool.tile([P, 1], F32)
            nc.vector.memset(eps_sb[:], EPS)
            sim_compat = bool(int(os.environ.get("KERNEL_SIM_COMPAT", "0")))

            def wslice(name, c0, n):
                return w_sb[:, _OFF[name] + c0:_OFF[name] + c0 + n]

            def bcol(name, j=0):
                return b_sb[:, _BOFF[name] + j:_BOFF[name] + j + 1]

            # per-relation activation buffers, split into scene-halves so
            # phase C's first half can start before the last scenes finish
            oT_all = {r: [core.tile([P, B_LOC * P // 2], BF16,
                                    tag=f"oT_{r}{i}", name=f"oT_{r}{i}")
                          for i in range(2)]
                      for r, _, _ in RELS}
            agT_all = [core.tile([P, B_LOC * P // 2], BF16, tag=f"agT{i}",
                                 name=f"agT{i}") for i in range(2)]
            hT_all = core.tile([P, B_LOC * P], F32, tag="hT")
            hnT_all = core.tile([P, B_LOC * P], BF16, tag="hnT")
            g_all = core.tile([P, 4, B_LOC * P], BF16, tag="g")
            outT_all = core.tile([P, B_LOC * P], F32, tag="outT")

            def ln_stats(mv, st6, x_sb, T, base):
                """bn stats for T tiles of x_sb into mv[:, base:base+T, :]."""
                for t in range(T):
                    nc.vector.bn_stats(st6[:, base + t, :], x_sb[:, t, :])
                    nc.vector.bn_aggr(mv[:, base + t, :], st6[:, base + t, :])

            def ln_rstd(rstd, mv, n):
                """rstd[:, :n] = 1/sqrt(var+eps), one batched ACT pass.

                On HW: exp(-0.5*ln(var+eps)) keeps the ACT engine in the
                natural_log_exp table (no table-load thrash against the
                attention exps).  CoreSim lacks Ln, so a sim-compat build
                uses Sqrt + DVE reciprocal instead."""
                if sim_compat:
                    nc.scalar.activation(rstd[:, :n], mv[:, :n, 1], AF.Sqrt,
                                         bias=eps_sb[:])
                    nc.vector.reciprocal(rstd[:, :n], rstd[:, :n])
                else:
                    nc.scalar.activation(rstd[:, :n], mv[:, :n, 1], AF.Ln,
                                         bias=eps_sb[:])
                    nc.scalar.activation(rstd[:, :n], rstd[:, :n], AF.Exp,
                                         scale=-0.5)

            def ln_apply(xn_dst, x_sb, mv, rstd, T, base):
                # apply on the (otherwise idle) Pool engine: 1-tensor-input op
                for t in range(T):
                    nc.gpsimd.tensor_scalar(
                        xn_dst[:, t, :], x_sb[:, t, :],
                        mv[:, base + t, 0:1], rstd[:, base + t:base + t + 1],
                        ALU.subtract, ALU.mult)

            _evac_flip = [0]

            def transpose_128(dst_sb, src_sb, ident, dtype, evac=None):
                """dst_sb[128,128] = src_sb[128,128].T via PE (through PSUM).

                The PSUM evacuation copy alternates DVE / ACT for balance
                unless an engine is forced via `evac`.
                """
                pt = ptq.tile([P, P], dtype, tag="pt")
                nc.tensor.transpose(pt[:], src_sb[:], ident)
                if evac is None:
                    _evac_flip[0] ^= 1
                    evac = nc.vector if _evac_flip[0] else nc.scalar
                if evac is nc.scalar:
                    nc.scalar.copy(dst_sb, pt[:])
                else:
                    nc.vector.tensor_copy(dst_sb, pt[:])

            N_TILES = sum(T for _, _, T in RELS)   # 13
            for s in range(B_LOC):
                # ---- phase A: LN + projections -----------------------------
                xnT = {}
                v_aug = {}
                kT = {}
                qT = act.tile([P, 3, P], BF16, tag="qT")
                # DMA + LN stats for all sources, one batched rstd pass
                x_sbs = {}
                st6 = small.tile([P, N_TILES, 6], F32, tag="st6")
                mv = small.tile([P, N_TILES, 2], F32, tag="mv")
                rstd = small.tile([P, N_TILES], F32, tag="rstd")
                base = 0
                bases = {}
                for r, NK, T in RELS:
                    x_sb = inp.tile([P, T, P], F32, tag=f"x_{r}")
                    nc.sync.dma_start(
                        x_sb[:], srcs[r][s].rearrange("(t p) d -> p t d", p=P))
                    x_sbs[r] = x_sb
                    ln_stats(mv, st6, x_sb, T, base)
                    bases[r] = base
                    base += T
                ln_rstd(rstd, mv, N_TILES)
                for r, NK, T in RELS:
                    x_sb = x_sbs[r]
                    xn = act.tile([P, T, P], BF16, tag=f"xn_{r}")
                    ln_apply(xn, x_sb, mv, rstd, T, bases[r])
                    # transpose each token tile -> xnT (feature-major)
                    xT = act.tile([P, T, P], BF16, tag=f"xT_{r}")
                    for t in range(T):
                        transpose_128(xT[:, t, :], xn[:, t, :], id_bf[:], BF16)
                    xnT[r] = xT
                    # k^T = Wk.T @ xn^T   (all tiles in <=512 chunks)
                    kr = act.tile([P, T * P], BF16, tag=f"kT_{r}")
                    kvkey = {"a2a": "kv_o", "l2a": "kv_l", "g2a": "kv_g"}[r]
                    wk = wslice(kvkey, 0, P)
                    for c0 in range(0, T * P, 512):
                        cw = min(512, T * P - c0)
                        pk = mm.tile([P, 512], F32, tag="mm")
                        nc.tensor.matmul(
                            pk[:, :cw], wk,
                            xT.rearrange("p t d -> p (t d)")[:, c0:c0 + cw])
                        if (c0 // 512) % 2 == 0:
                            nc.vector.tensor_copy(kr[:, c0:c0 + cw], pk[:, :cw])
                        else:
                            nc.scalar.copy(kr[:, c0:c0 + cw], pk[:, :cw])
                    kT[r] = kr
                    # v = xn @ Wv, ones column interleaved: v_aug[128,T,4,33]
                    wv = wslice(kvkey, P, P)
                    va = act.tile([P, T, 4, 33], BF16, tag=f"va_{r}")
                    # ones columns for the fused softmax denominator (same-size
                    # bitcast, so the strided AP is fine for memset)
                    nc.gpsimd.memset(va[:, :, :, 32:33], 1.0)
                    for t in range(T):
                        pv = mm.tile([P, 512], F32, tag="mm")
                        nc.tensor.matmul(pv[:, :P], xT[:, t, :], wv)
                        vsrc = pv[:, :P].rearrange("p (h d) -> p h d", h=H)
                        if t % 2 == 0:
                            nc.scalar.copy(va[:, t, :, 0:32], vsrc)
                        else:
                            nc.vector.tensor_copy(va[:, t, :, 0:32], vsrc)
                    v_aug[r] = va
                    if r == "a2a":
                        # q^T (3 relation blocks) from the agent xn^T
                        for j in range(3):
                            pq = mm.tile([P, 512], F32, tag="mm")
                            nc.tensor.matmul(pq[:, :P], wslice("wq", j * P, P),
                                             xT[:, 0, :])
                            nc.vector.tensor_scalar(
                                qT[:, j, :], pq[:, :P], bcol("bq", j), None,
                                ALU.add)
                        # raw agent^T (bf16) for self_fc
                        xraw = act.tile([P, P], BF16, tag="xraw")
                        nc.vector.tensor_copy(xraw[:], x_sb[:, 0, :])
                        transpose_128(
                            agT_all[s // 4][:, (s % 4) * P:(s % 4 + 1) * P],
                            xraw[:], id_bf[:], BF16)

                # ---- phase B: attention ------------------------------------
                for ri, (r, NK, T) in enumerate(RELS):
                    nm_sb = act.tile([P, T, P], BF16, tag=f"nm_{r}")
                    nc.sync.dma_start(
                        nm_sb[:], nms[r][s].rearrange("p (t q) -> p t q", t=T))
                    po = pop.tile([P, H * 33], F32, tag="po")
                    kT3 = kT[r].rearrange("p (t d) -> p t d", d=P)
                    for h in range(H):
                        hs = slice(h * DK, (h + 1) * DK)
                        # all score tiles for this head first (keeps the PE in
                        # 32-row tiling mode), then all o-matmuls (full mode)
                        pt = ptp.tile([P, 8, P], BF16, tag="pT", name="pT")
                        for t0 in range(0, T, 4):
                            g = min(4, T - t0)
                            ps = mm.tile([P, 4, P], F32, tag="mm", name="ps")
                            for j in range(g):
                                nc.tensor.matmul(
                                    ps[:, j, :], kT3[hs, t0 + j, :],
                                    qT[hs, ri, :],
                                    tile_position=(h * DK, 0))
                            nc.scalar.activation(pt[:, t0:t0 + g, :],
                                                 ps[:, :g, :], AF.Exp)
                            # mask multiply: mostly DVE (bf16 2x), 1/4 to Pool
                            eng = nc.gpsimd if (t0 // 4 + h) % 2 == 1 else nc.vector
                            eng.tensor_mul(pt[:, t0:t0 + g, :],
                                           pt[:, t0:t0 + g, :],
                                           nm_sb[:, t0:t0 + g, :])
                        for t in range(T):
                            nc.tensor.matmul(
                                po[:, h * 33:(h + 1) * 33],
                                pt[:, t, :],
                                v_aug[r][:, t, h, :],
                                start=(t == 0), stop=(t == T - 1))
                    # normalize + evacuate o, then transpose to oT_all
                    po3 = po.rearrange("p (h c) -> p h c", c=33)
                    rcp = small.tile([P, H, 1], F32, tag="rcp")
                    nc.vector.reciprocal(rcp[:, :, 0], po3[:, :, 32])
                    o_sb = act.tile([P, P], BF16, tag="o_sb")
                    nc.vector.tensor_tensor(
                        o_sb.rearrange("p (h d) -> p h d", h=H),
                        po3[:, :, 0:32],
                        rcp[:].to_broadcast((P, H, DK)),
                        ALU.mult)
                    transpose_128(
                        oT_all[r][s // 4][:, (s % 4) * P:(s % 4 + 1) * P],
                        o_sb[:], id_bf[:], BF16)

            # ---- phase C: fc / out_fc / FFN over all scenes ----------------
            QW = B_LOC * P     # 1024 moving width
            fc_in = [agT_all, oT_all["a2a"], oT_all["l2a"], oT_all["g2a"]]
            # cat order in reference: [h_self, h_other, h_l2a, h_g2a]
            hT = [[core.tile([P, QW // 2], BF16, tag=f"hT_{j}{i}",
                              name=f"hT_{j}{i}") for i in range(2)]
                  for j in range(4)]
            for half in range(2):
                for j in range(4):
                    pf = mm.tile([P, 512], F32, tag="mm")
                    nc.tensor.matmul(pf[:], wslice("wfc", j * P, P),
                                     fc_in[j][half][:])
                    nc.scalar.activation(hT[j][half][:], pf[:],
                                         AF.Relu, bias=bcol("bfc", j))
                ph = mm.tile([P, 512], F32, tag="mm")
                for j in range(4):
                    nc.tensor.matmul(ph[:], wslice("wout", j * P, P),
                                     hT[j][half][:],
                                     start=(j == 0), stop=(j == 3))
                nc.vector.tensor_scalar(hT_all[:, half * 512:(half + 1) * 512],
                                        ph[:], bcol("bout"), None, ALU.add)
            # FFN layernorm (token-major transpose round trip); stats for all
            # scenes first so the rstd ACT pass is batched once
            h_tok = core.tile([P, B_LOC, P], F32, tag="h_tok")
            st6f = small.tile([P, B_LOC, 6], F32, tag="st6")
            mvf = small.tile([P, B_LOC, 2], F32, tag="mv")
            rstdf = small.tile([P, B_LOC], F32, tag="rstd")
            for s in range(B_LOC):
                transpose_128(h_tok[:, s, :], hT_all[:, s * P:(s + 1) * P],
                              id_f32[:], F32)
                ln_stats(mvf, st6f, h_tok[:, s:s + 1, :], 1, s)
            ln_rstd(rstdf, mvf, B_LOC)
            for s in range(B_LOC):
                hn = act.tile([P, P], BF16, tag="hn_tok")
                ln_apply(hn.rearrange("p (t d) -> p t d", t=1),
                         h_tok[:, s:s + 1, :], mvf, rstdf, 1, s)
                transpose_128(hnT_all[:, s * P:(s + 1) * P], hn[:],
                              id_bf[:], BF16)
            # w1 (silu) * w3 -> g ; w2 @ g + b2 + h -> outT
            for c0 in range(0, QW, 512):
                for c in range(4):
                    p1 = mm.tile([P, 512], F32, tag="mm")
                    nc.tensor.matmul(p1[:], wslice("w1", c * P, P),
                                     hnT_all[:, c0:c0 + 512])
                    # silu(a) = a * sigmoid(a): sigmoid on ACT, multiply fused
                    # into the PSUM evacuation (CoreSim lacks a Silu table)
                    sg = act.tile([P, 512], BF16, tag="sg")
                    nc.scalar.activation(sg[:], p1[:], AF.Sigmoid,
                                         bias=bcol("b1", c))
                    t1 = act.tile([P, 512], BF16, tag="t1")
                    nc.vector.scalar_tensor_tensor(
                        t1[:], p1[:], bcol("b1", c), sg[:], ALU.add, ALU.mult)
                    p3 = mm.tile([P, 512], F32, tag="mm")
                    nc.tensor.matmul(p3[:], wslice("w3", c * P, P),
                                     hnT_all[:, c0:c0 + 512])
                    nc.vector.scalar_tensor_tensor(
                        g_all[:, c, c0:c0 + 512], p3[:], bcol("b3", c), t1[:],
                        ALU.add, ALU.mult)
                pf = mm.tile([P, 512], F32, tag="mm")
                for c in range(4):
                    nc.tensor.matmul(pf[:], wslice("w2", c * P, P),
                                     g_all[:, c, c0:c0 + 512],
                                     start=(c == 0), stop=(c == 3))
                nc.vector.scalar_tensor_tensor(
                    outT_all[:, c0:c0 + 512], pf[:], bcol("b2"),
                    hT_all[:, c0:c0 + 512], ALU.add, ALU.add)
            for s in range(B_LOC):
                o_tok = act.tile([P, P], F32, tag="o_tok")
                transpose_128(o_tok[:], outT_all[:, s * P:(s + 1) * P],
                              id_f32[:], F32)
                nc.sync.dma_start(out[s], o_tok[:])

    # Constrain the act-table chooser so Exp and Ln resolve to the ONE table
    # that holds both (natural_log_exp_and_others): without this, Exp picks
    # exp_and_others and Ln picks natural_log and every LN<->attention
    # transition pays a ~1.3us ACT table load (61 loads -> ~78us/core).
    # Table positions (= act_func_set ids walrus consumes) are unchanged;
    # we only shrink the competing sets the chooser may select.
    import concourse.bacc as bacc_mod
    _orig_tables = bacc_mod.get_activation_tables

    def _constrained(arch):
        t = {k: set(v) for k, v in _orig_tables(arch).items()}
        for name, s in t.items():
            if name != "natural_log_exp_and_others":
                s.discard(AF.Exp)
                s.discard(AF.Ln)
        return t

    bacc_mod.get_activation_tables = _constrained
    try:
        nc.compile()
    finally:
        bacc_mod.get_activation_tables = _orig_tables
    return nc


_PROG = None


def _get_program():
    global _PROG
    if _PROG is None:
        _PROG = _build_program()
    return _PROG


def _prep_host(params):
    """Fold LN affine params / attention scales / v-biases into weights.

    Returns (wpack[128, W_COLS] bf16, bpack[128, B_COLS] f32).
    """
    import math as _m

    def _np(v):
        if isinstance(v, (tuple, list)):
            return tuple(np.asarray(x, np.float32) for x in v)
        return np.asarray(v, np.float32)

    g = {k: _np(v) for k, v in params.items()}

    def fold(ln, w):
        gam, bet = ln
        return gam[:, None] * w, bet @ w

    wq, bq = fold(g["ln_q"], g["wq"])             # [128,384], [384]
    # attention scale folded into each relation's q block
    for j, nk in enumerate([NA, NL, NG]):
        sc = _m.log(nk + 1.0, 32) / _m.sqrt(DK)
        wq[:, j * P:(j + 1) * P] *= sc
        bq[j * P:(j + 1) * P] *= sc
    kv = {}
    vbias = {}
    for r, lnn, wn in [("o", "ln_other", "wkv_other"),
                       ("l", "ln_l2a", "wkv_l2a"),
                       ("g", "ln_g2a", "wkv_g2a")]:
        w, b = fold(g[lnn], g[wn])                # [128,256], [256]
        kv[r] = w
        kv[r + "_kb"] = b[:P]                      # k bias (zero for this model)
        vbias[r] = b[P:]                           # v bias -> folded into fc bias
    # fc weights / biases in device order (self, other, l2a, g2a), with the
    # v-bias fold: relu((o + vb) @ W + b) = relu(o @ W + (vb @ W + b))
    wfc = np.concatenate([g["self_fc"][0], g["fc_other"][0], g["fc_l2a"][0],
                          g["fc_g2a"][0]], axis=1)         # [128, 4*128]
    bfc = np.stack([
        g["self_fc"][1],
        g["fc_other"][1] + vbias["o"] @ g["fc_other"][0],
        g["fc_l2a"][1] + vbias["l"] @ g["fc_l2a"][0],
        g["fc_g2a"][1] + vbias["g"] @ g["fc_g2a"][0]], axis=1)
    # out_fc: cat order [self, other, l2a, g2a] rows of wout
    wout, bout = g["out_fc"]                       # [512,128],[128]
    wout_p = wout.reshape(4, P, P).transpose(1, 0, 2).reshape(P, 4 * P)
    # ffn
    w1, b1 = g["ffn_w1"]
    w2, b2 = g["ffn_w2"]
    w3, b3 = g["ffn_w3"]
    gam, bet = g["ffn_ln"]
    w1f = gam[:, None] * w1
    b1f = bet @ w1 + b1
    w3f = gam[:, None] * w3
    b3f = bet @ w3 + b3
    w2_p = w2.reshape(4, P, P).transpose(1, 0, 2).reshape(P, 4 * P)

    wpack = np.zeros((P, W_COLS), np.float32)
    wpack[:, _OFF["wq"]:_OFF["wq"] + 384] = wq
    for r, key in [("o", "kv_o"), ("l", "kv_l"), ("g", "kv_g")]:
        wpack[:, _OFF[key]:_OFF[key] + 256] = kv[r]
    wpack[:, _OFF["wfc"]:_OFF["wfc"] + 512] = wfc
    wpack[:, _OFF["wout"]:_OFF["wout"] + 512] = wout_p
    wpack[:, _OFF["w1"]:_OFF["w1"] + 512] = w1f
    wpack[:, _OFF["w3"]:_OFF["w3"] + 512] = w3f
    wpack[:, _OFF["w2"]:_OFF["w2"] + 512] = w2_p

    bpack = np.zeros((P, B_COLS), np.float32)
    bpack[:, _BOFF["bq"]:_BOFF["bq"] + 3] = bq.reshape(3, P).T
    bpack[:, _BOFF["bfc"]:_BOFF["bfc"] + 4] = bfc
    bpack[:, _BOFF["bout"]] = bout
    bpack[:, _BOFF["b1"]:_BOFF["b1"] + 4] = b1f.reshape(4, P).T
    bpack[:, _BOFF["b3"]:_BOFF["b3"] + 4] = b3f.reshape(4, P).T
    bpack[:, _BOFF["b2"]] = b2

    import ml_dtypes
    return wpack.astype(ml_dtypes.bfloat16), bpack


def _prep_mask(mask):
    """bool [Bl, NA, NK] -> notmask^T bf16 [Bl, 128, T*NA] (k-partition-major)."""
    import ml_dtypes
    bl, na, nk = mask.shape
    t = nk // P
    nmT = (~np.asarray(mask, bool)).astype(np.float32).transpose(0, 2, 1)
    nmT = nmT.reshape(bl, t, P, na).transpose(0, 2, 1, 3).reshape(bl, P, t * na)
    return np.ascontiguousarray(nmT.astype(ml_dtypes.bfloat16))


def kernel(agent_h, lane_h, poly_h, a2a_mask, l2a_mask, g2a_mask, params):
    agent_h = np.ascontiguousarray(np.asarray(agent_h, np.float32))
    lane_h = np.ascontiguousarray(np.asarray(lane_h, np.float32))
    poly_h = np.ascontiguousarray(np.asarray(poly_h, np.float32))
    wpack, bpack = _prep_host(params)
    nm_a = _prep_mask(np.asarray(a2a_mask))
    nm_l = _prep_mask(np.asarray(l2a_mask))
    nm_g = _prep_mask(np.asarray(g2a_mask))

    nc = _get_program()
    in_maps = []
    for c in range(N_CORES):
        sl = slice(c * B_LOC, (c + 1) * B_LOC)
        in_maps.append({
            "agent": agent_h[sl], "lane": lane_h[sl], "poly": poly_h[sl],
            "nm_a": nm_a[sl], "nm_l": nm_l[sl], "nm_g": nm_g[sl],
            "wpack": wpack, "bpack": bpack,
        })
    trace = bool(int(os.environ.get("KERNEL_TRACE", "0")))
    res = run_bass_kernel_spmd(nc, in_maps, list(range(N_CORES)), trace=trace)
    if trace and res.exec_time_ns is not None:
        print(f"HW exec time: {res.exec_time_ns} ns")
    out = np.concatenate([res.results[c]["out"] for c in range(N_CORES)], axis=0)
    return out.astype(np.float32)


# revision 49
# speedup vs baseline: 12600.8961x; 1.0008x over previous
"""AgentHetGNN layer on 8 Trainium2 NeuronCores (Bass/Tile, SPMD data-parallel).

Strategy
--------
Data-parallel over the 64 scenes: 8 scenes per core.  Inside a core,
per scene:
  A) LayerNorm (stats on DVE, apply fused sub+mul) + projections.
     Activations move feature-major ("^T", [feat, token]) so matmul
     contractions land on partitions; per-128-token tiles are PE-transposed.
     LN gammas are folded into the projection weights on the host, the
     attention scale is folded into wq, v-projection biases are folded
     into the post-attention fc biases (all exact algebra).
  B) Attention per (relation, head, key-tile): scores are produced
     TRANSPOSED (s^T = k_h^T.T @ q_h^T, K=32 contraction), exp on ACT
     (PSUM->SBUF, bf16), multiplicative mask (notmask, bf16, DVE 4x mode),
     then o = p^T.T @ [v | 1] with a ones-column interleaved into v so the
     softmax denominator accumulates for free in the same PSUM tile.
     No max-subtraction is needed: scores are O(1) by construction and
     masked entries are exactly zeroed after exp.
  C) fc/out_fc/FFN batched across all 8 scenes per core (moving operand
     width 512-1024), silu on ACT, fused (psum+bias)*t / (psum+bias)+h
     on DVE via scalar_tensor_tensor.

All heavy matmuls run in bf16 (1 cyc/row on PE); f32 is kept for inputs,
LN statistics, PSUM accumulation, the residual h, and the final output.
"""

import os
import numpy as np

import concourse.bass as bass
import concourse.mybir as mybir
import concourse.tile as tile
from concourse.bass_utils import run_bass_kernel_spmd
from concourse.masks import make_identity

F32 = mybir.dt.float32
BF16 = mybir.dt.bfloat16
AF = mybir.ActivationFunctionType
ALU = mybir.AluOpType

N_CORES = 8
B = 64
B_LOC = B // N_CORES          # scenes per core
NA, NL, NG, D = 128, 1024, 512, 128
H, DK = 4, 32
P = 128
EPS = 1e-5

# relations: (name, n_key_tokens, tiles-of-128)
RELS = [("a2a", NA, NA // P), ("l2a", NL, NL // P), ("g2a", NG, NG // P)]

# ---------------------------------------------------------------------------
# weight packing offsets (columns in the single packed bf16 weight tensor)
# layout: wq(384) | k_o(128) v_o(128) | k_l(128) v_l(128) | k_g(128) v_g(128)
#         | wfc(4*128) | wout(4*128) | w1(512) | w3(512) | w2(4*128)
_OFF = {}
_c = 0
for _name, _w in [("wq", 384), ("kv_o", 256), ("kv_l", 256), ("kv_g", 256),
                  ("wfc", 512), ("wout", 512), ("w1", 512), ("w3", 512),
                  ("w2", 512)]:
    _OFF[_name] = _c
    _c += _w
W_COLS = _c

# bias packing: bq(3) | bfc(4) | bout(1) | b1(4) | b3(4) | b2(1)
_BOFF = {"bq": 0, "bfc": 3, "bout": 7, "b1": 8, "b3": 12, "b2": 16}
B_COLS = 17


def _build_program():
    from concourse import bacc
    nc = bacc.Bacc(None)

    agent = nc.declare_dram_parameter("agent", [B_LOC, NA, D], F32, isOutput=False)
    lane = nc.declare_dram_parameter("lane", [B_LOC, NL, D], F32, isOutput=False)
    poly = nc.declare_dram_parameter("poly", [B_LOC, NG, D], F32, isOutput=False)
    nm_a = nc.declare_dram_parameter("nm_a", [B_LOC, P, (NA // P) * NA], BF16, isOutput=False)
    nm_l = nc.declare_dram_parameter("nm_l", [B_LOC, P, (NL // P) * NA], BF16, isOutput=False)
    nm_g = nc.declare_dram_parameter("nm_g", [B_LOC, P, (NG // P) * NA], BF16, isOutput=False)
    wpack = nc.declare_dram_parameter("wpack", [P, W_COLS], BF16, isOutput=False)
    bpack = nc.declare_dram_parameter("bpack", [P, B_COLS], F32, isOutput=False)
    out = nc.declare_dram_parameter("out", [B_LOC, NA, D], F32, isOutput=True)

    srcs = {"a2a": agent, "l2a": lane, "g2a": poly}
    nms = {"a2a": nm_a, "l2a": nm_l, "g2a": nm_g}

    with tile.TileContext(nc) as tc:
        from contextlib import ExitStack
        with ExitStack() as ctx:
            cpool = ctx.enter_context(tc.tile_pool(name="const", bufs=1))
            inp = ctx.enter_context(tc.tile_pool(name="inp", bufs=3))
            act = ctx.enter_context(tc.tile_pool(name="act", bufs=5))
            small = ctx.enter_context(tc.tile_pool(name="small", bufs=6))
            ptp = ctx.enter_context(tc.tile_pool(name="ptp", bufs=6))
            core = ctx.enter_context(tc.tile_pool(name="core", bufs=1))
            mm = ctx.enter_context(tc.tile_pool(name="mm", bufs=3, space="PSUM"))
            pop = ctx.enter_context(tc.tile_pool(name="pop", bufs=2, space="PSUM"))
            ptq = ctx.enter_context(tc.tile_pool(name="ptq", bufs=3, space="PSUM"))

            # ---- constants -------------------------------------------------
            w_sb = cpool.tile([P, W_COLS], BF16)
            nc.sync.dma_start(w_sb[:], wpack[:])
            b_sb = cpool.tile([P, B_COLS], F32)
            nc.sync.dma_start(b_sb[:], bpack[:])
            id_bf = cpool.tile([P, P], BF16)
            make_identity(nc, id_bf[:])
            id_f32 = cpool.tile([P, P], F32)
            make_identity(nc, id_f32[:])
            eps_sb = cpool.tile([P, 1], F32)
            nc.vector.memset(eps_sb[:], EPS)
            sim_compat = bool(int(os.environ.get("KERNEL_SIM_COMPAT", "0")))

            def wslice(name, c0, n):
                return w_sb[:, _OFF[name] + c0:_OFF[name] + c0 + n]

            def bcol(name, j=0):
                return b_sb[:, _BOFF[name] + j:_BOFF[name] + j + 1]

            # per-relation activation buffers, split into scene-halves so
            # phase C's first half can start before the last scenes finish
            oT_all = {r: [core.tile([P, B_LOC * P // 2], BF16,
                                    tag=f"oT_{r}{i}", name=f"oT_{r}{i}")
                          for i in range(2)]
                      for r, _, _ in RELS}
            agT_all = [core.tile([P, B_LOC * P // 2], BF16, tag=f"agT{i}",
                                 name=f"agT{i}") for i in range(2)]
            hT_all = core.tile([P, B_LOC * P], F32, tag="hT")
            hnT_all = core.tile([P, B_LOC * P], BF16, tag="hnT")
            g_all = core.tile([P, 4, B_LOC * P], BF16, tag="g")
            outT_all = core.tile([P, B_LOC * P], F32, tag="outT")

            def ln_stats(mv, st6, x_sb, T, base):
                """bn stats for T tiles of x_sb into mv[:, base:base+T, :]."""
                for t in range(T):
                    nc.vector.bn_stats(st6[:, base + t, :], x_sb[:, t, :])
                    nc.vector.bn_aggr(mv[:, base + t, :], st6[:, base + t, :])

            def ln_rstd(rstd, mv, n):
                """rstd[:, :n] = 1/sqrt(var+eps), one batched ACT pass.

                On HW: exp(-0.5*ln(var+eps)) keeps the ACT engine in the
                natural_log_exp table (no table-load thrash against the
                attention exps).  CoreSim lacks Ln, so a sim-compat build
                uses Sqrt + DVE reciprocal instead."""
                if sim_compat:
                    nc.scalar.activation(rstd[:, :n], mv[:, :n, 1], AF.Sqrt,
                                         bias=eps_sb[:])
                    nc.vector.reciprocal(rstd[:, :n], rstd[:, :n])
                else:
                    nc.scalar.activation(rstd[:, :n], mv[:, :n, 1], AF.Ln,
                                         bias=eps_sb[:])
                    nc.scalar.activation(rstd[:, :n], rstd[:, :n], AF.Exp,
                                         scale=-0.5)

            def ln_apply(xn_dst, x_sb, mv, rstd, T, base):
                # apply on the (otherwise idle) Pool engine: 1-tensor-input op
                for t in range(T):
                    nc.gpsimd.tensor_scalar(
                        xn_dst[:, t, :], x_sb[:, t, :],
                        mv[:, base + t, 0:1], rstd[:, base + t:base + t + 1],
                        ALU.subtract, ALU.mult)

            _evac_flip = [0]

            def transpose_128(dst_sb, src_sb, ident, dtype, evac=None):
                """dst_sb[128,128] = src_sb[128,128].T via PE (through PSUM).

                The PSUM evacuation copy alternates DVE / ACT for balance
                unless an engine is forced via `evac`.
                """
                pt = ptq.tile([P, P], dtype, tag="pt")
                nc.tensor.transpose(pt[:], src_sb[:], ident)
                if evac is None:
                    _evac_flip[0] ^= 1
                    evac = nc.vector if _evac_flip[0] else nc.scalar
                if evac is nc.scalar:
                    nc.scalar.copy(dst_sb, pt[:])
                else:
                    nc.vector.tensor_copy(dst_sb, pt[:])

            N_TILES = sum(T for _, _, T in RELS)   # 13
            for s in range(B_LOC):
                # ---- phase A: LN + projections -----------------------------
                xnT = {}
                v_aug = {}
                kT = {}
                qT = act.tile([P, 3, P], BF16, tag="qT")
                # DMA + LN stats for all sources, one batched rstd pass
                x_sbs = {}
                st6 = small.tile([P, N_TILES, 6], F32, tag="st6")
                mv = small.tile([P, N_TILES, 2], F32, tag="mv")
                rstd = small.tile([P, N_TILES], F32, tag="rstd")
                base = 0
                bases = {}
                for r, NK, T in RELS:
                    x_sb = inp.tile([P, T, P], F32, tag=f"x_{r}")
                    nc.sync.dma_start(
                        x_sb[:], srcs[r][s].rearrange("(t p) d -> p t d", p=P))
                    x_sbs[r] = x_sb
                    ln_stats(mv, st6, x_sb, T, base)
                    bases[r] = base
                    base += T
                ln_rstd(rstd, mv, N_TILES)
                for r, NK, T in RELS:
                    x_sb = x_sbs[r]
                    xn = act.tile([P, T, P], BF16, tag=f"xn_{r}")
                    ln_apply(xn, x_sb, mv, rstd, T, bases[r])
                    # transpose each token tile -> xnT (feature-major)
                    xT = act.tile([P, T, P], BF16, tag=f"xT_{r}")
                    for t in range(T):
                        transpose_128(xT[:, t, :], xn[:, t, :], id_bf[:], BF16)
                    xnT[r] = xT
                    # k^T = Wk.T @ xn^T   (all tiles in <=512 chunks)
                    kr = act.tile([P, T * P], BF16, tag=f"kT_{r}")
                    kvkey = {"a2a": "kv_o", "l2a": "kv_l", "g2a": "kv_g"}[r]
                    wk = wslice(kvkey, 0, P)
                    for c0 in range(0, T * P, 512):
                        cw = min(512, T * P - c0)
                        pk = mm.tile([P, 512], F32, tag="mm")
                        nc.tensor.matmul(
                            pk[:, :cw], wk,
                            xT.rearrange("p t d -> p (t d)")[:, c0:c0 + cw])
                        if (c0 // 512) % 2 == 0:
                            nc.vector.tensor_copy(kr[:, c0:c0 + cw], pk[:, :cw])
                        else:
                            nc.scalar.copy(kr[:, c0:c0 + cw], pk[:, :cw])
                    kT[r] = kr
                    # v = xn @ Wv, ones column interleaved: v_aug[128,T,4,33]
                    wv = wslice(kvkey, P, P)
                    va = act.tile([P, T, 4, 33], BF16, tag=f"va_{r}")
                    # ones columns for the fused softmax denominator (same-size
                    # bitcast, so the strided AP is fine for memset)
                    nc.gpsimd.memset(va[:, :, :, 32:33], 1.0)
                    for t in range(T):
                        pv = mm.tile([P, 512], F32, tag="mm")
                        nc.tensor.matmul(pv[:, :P], xT[:, t, :], wv)
                        vsrc = pv[:, :P].rearrange("p (h d) -> p h d", h=H)
                        if t % 2 == 0:
                            nc.scalar.copy(va[:, t, :, 0:32], vsrc)
                        else:
                            nc.vector.tensor_copy(va[:, t, :, 0:32], vsrc)
                    v_aug[r] = va
                    if r == "a2a":
                        # q^T (3 relation blocks) from the agent xn^T
                        for j in range(3):
                            pq = mm.tile([P, 512], F32, tag="mm")
                            nc.tensor.matmul(pq[:, :P], wslice("wq", j * P, P),
                                             xT[:, 0, :])
                            nc.vector.tensor_scalar(
                                qT[:, j, :], pq[:, :P], bcol("bq", j), None,
                                ALU.add)
                        # raw agent^T (bf16) for self_fc
                        xraw = act.tile([P, P], BF16, tag="xraw")
                        nc.vector.tensor_copy(xraw[:], x_sb[:, 0, :])
                        transpose_128(
                            agT_all[s // 4][:, (s % 4) * P:(s % 4 + 1) * P],
                            xraw[:], id_bf[:], BF16)

                # ---- phase B: attention ------------------------------------
                for ri, (r, NK, T) in enumerate(RELS):
                    nm_sb = act.tile([P, T, P], BF16, tag=f"nm_{r}")
                    nc.sync.dma_start(
                        nm_sb[:], nms[r][s].rearrange("p (t q) -> p t q", t=T))
                    po = pop.tile([P, H * 33], F32, tag="po")
                    kT3 = kT[r].rearrange("p (t d) -> p t d", d=P)
                    for h in range(H):
                        hs = slice(h * DK, (h + 1) * DK)
                        # all score tiles for this head first (keeps the PE in
                        # 32-row tiling mode), then all o-matmuls (full mode)
                        pt = ptp.tile([P, 8, P], BF16, tag="pT", name="pT")
                        for t0 in range(0, T, 4):
                            g = min(4, T - t0)
                            ps = mm.tile([P, 4, P], F32, tag="mm", name="ps")
                            for j in range(g):
                                nc.tensor.matmul(
                                    ps[:, j, :], kT3[hs, t0 + j, :],
                                    qT[hs, ri, :],
                                    tile_position=(h * DK, 0))
                            nc.scalar.activation(pt[:, t0:t0 + g, :],
                                                 ps[:, :g, :], AF.Exp)
                            # mask multiply: mostly DVE (bf16 2x), 1/4 to Pool
                            eng = nc.gpsimd if (t0 // 4 + h) % 2 == 1 else nc.vector
                            eng.tensor_mul(pt[:, t0:t0 + g, :],
                                           pt[:, t0:t0 + g, :],
                                           nm_sb[:, t0:t0 + g, :])
                        for t in range(T):
                            nc.tensor.matmul(
                                po[:, h * 33:(h + 1) * 33],
                                pt[:, t, :],
                                v_aug[r][:, t, h, :],
                                start=(t == 0), stop=(t == T - 1))
                    # normalize + evacuate o, then transpose to oT_all
                    po3 = po.rearrange("p (h c) -> p h c", c=33)
                    rcp = small.tile([P, H, 1], F32, tag="rcp")
                    nc.vector.reciprocal(rcp[:, :, 0], po3[:, :, 32])
                    o_sb = act.tile([P, P], BF16, tag="o_sb")
                    nc.vector.tensor_tensor(
                        o_sb.rearrange("p (h d) -> p h d", h=H),
                        po3[:, :, 0:32],
                        rcp[:].to_broadcast((P, H, DK)),
                        ALU.mult)
                    transpose_128(
                        oT_all[r][s // 4][:, (s % 4) * P:(s % 4 + 1) * P],
                        o_sb[:], id_bf[:], BF16)

            # ---- phase C: fc / out_fc / FFN over all scenes ----------------
            QW = B_LOC * P     # 1024 moving width
            fc_in = [agT_all, oT_all["a2a"], oT_all["l2a"], oT_all["g2a"]]
            # cat order in reference: [h_self, h_other, h_l2a, h_g2a]
            hT = [[core.tile([P, QW // 2], BF16, tag=f"hT_{j}{i}",
                              name=f"hT_{j}{i}") for i in range(2)]
                  for j in range(4)]
            for half in range(2):
                for j in range(4):
                    pf = mm.tile([P, 512], F32, tag="mm")
                    nc.tensor.matmul(pf[:], wslice("wfc", j * P, P),
                                     fc_in[j][half][:])
                    nc.scalar.activation(hT[j][half][:], pf[:],
                                         AF.Relu, bias=bcol("bfc", j))
                ph = mm.tile([P, 512], F32, tag="mm")
                for j in range(4):
                    nc.tensor.matmul(ph[:], wslice("wout", j * P, P),
                                     hT[j][half][:],
                                     start=(j == 0), stop=(j == 3))
                nc.vector.tensor_scalar(hT_all[:, half * 512:(half + 1) * 512],
                                        ph[:], bcol("bout"), None, ALU.add)
            # FFN layernorm (token-major transpose round trip); stats for all
            # scenes first so the rstd ACT pass is batched once
            h_tok = core.tile([P, B_LOC, P], F32, tag="h_tok")
            st6f = small.tile([P, B_LOC, 6], F32, tag="st6")
            mvf = small.tile([P, B_LOC, 2], F32, tag="mv")
            rstdf = small.tile([P, B_LOC], F32, tag="rstd")
            for s in range(B_LOC):
                transpose_128(h_tok[:, s, :], hT_all[:, s * P:(s + 1) * P],
                              id_f32[:], F32)
                ln_stats(mvf, st6f, h_tok[:, s:s + 1, :], 1, s)
            ln_rstd(rstdf, mvf, B_LOC)
            for s in range(B_LOC):
                hn = act.tile([P, P], BF16, tag="hn_tok")
                ln_apply(hn.rearrange("p (t d) -> p t d", t=1),
                         h_tok[:, s:s + 1, :], mvf, rstdf, 1, s)
                transpose_128(hnT_all[:, s * P:(s + 1) * P], hn[:],
                              id_bf[:], BF16)
            # w1 (silu) * w3 -> g ; w2 @ g + b2 + h -> outT
            for c0 in range(0, QW, 512):
                for c in range(4):
                    p1 = mm.tile([P, 512], F32, tag="mm")
                    nc.tensor.matmul(p1[:], wslice("w1", c * P, P),
                                     hnT_all[:, c0:c0 + 512])
                    # silu(a) = a * sigmoid(a): sigmoid on ACT, multiply fused
                    # into the PSUM evacuation (CoreSim lacks a Silu table)
                    sg = act.tile([P, 512], BF16, tag="sg")
                    nc.scalar.activation(sg[:], p1[:], AF.Sigmoid,
                                         bias=bcol("b1", c))
                    t1 = act.tile([P, 512], BF16, tag="t1")
                    nc.vector.scalar_tensor_tensor(
                        t1[:], p1[:], bcol("b1", c), sg[:], ALU.add, ALU.mult)
                    p3 = mm.tile([P, 512], F32, tag="mm")
                    nc.tensor.matmul(p3[:], wslice("w3", c * P, P),
                                     hnT_all[:, c0:c0 + 512])
                    nc.vector.scalar_tensor_tensor(
                        g_all[:, c, c0:c0 + 512], p3[:], bcol("b3", c), t1[:],
                        ALU.add, ALU.mult)
                pf = mm.tile([P, 512], F32, tag="mm")
                for c in range(4):
                    nc.tensor.matmul(pf[:], wslice("w2", c * P, P),
                                     g_all[:, c, c0:c0 + 512],
                                     start=(c == 0), stop=(c == 3))
                nc.vector.scalar_tensor_tensor(
                    outT_all[:, c0:c0 + 512], pf[:], bcol("b2"),
                    hT_all[:, c0:c0 + 512], ALU.add, ALU.add)
            for s in range(B_LOC):
                o_tok = act.tile([P, P], F32, tag="o_tok")
                transpose_128(o_tok[:], outT_all[:, s * P:(s + 1) * P],
                              id_f32[:], F32)
                nc.sync.dma_start(out[s], o_tok[:])

    # Constrain the act-table chooser so Exp and Ln resolve to the ONE table
    # that holds both (natural_log_exp_and_others): without this, Exp picks
    # exp_and_others and Ln picks natural_log and every LN<->attention
    # transition pays a ~1.3us ACT table load (61 loads -> ~78us/core).
    # Table positions (= act_func_set ids walrus consumes) are unchanged;
    # we only shrink the competing sets the chooser may select.
    import concourse.bacc as bacc_mod
    _orig_tables = bacc_mod.get_activation_tables

    def _constrained(arch):
        t = {k: set(v) for k, v in _orig_tables(arch).items()}
        for name, s in t.items():
            if name != "natural_log_exp_and_others":
                s.discard(AF.Exp)
                s.discard(AF.Ln)
        return t

    bacc_mod.get_activation_tables = _constrained
    try:
        nc.compile()
    finally:
        bacc_mod.get_activation_tables = _orig_tables
    return nc


_PROG = None


def _get_program():
    global _PROG
    if _PROG is None:
        _PROG = _build_program()
    return _PROG


def _prep_host(params):
    """Fold LN affine params / attention scales / v-biases into weights.

    Returns (wpack[128, W_COLS] bf16, bpack[128, B_COLS] f32).
    """
    import math as _m

    def _np(v):
        if isinstance(v, (tuple, list)):
            return tuple(np.asarray(x, np.float32) for x in v)
        return np.asarray(v, np.float32)

    g = {k: _np(v) for k, v in params.items()}

    def fold(ln, w):
        gam, bet = ln
        return gam[:, None] * w, bet @ w

    wq, bq = fold(g["ln_q"], g["wq"])             # [128,384], [384]
    # attention scale folded into each relation's q block
    for j, nk in enumerate([NA, NL, NG]):
        sc = _m.log(nk + 1.0, 32) / _m.sqrt(DK)
        wq[:, j * P:(j + 1) * P] *= sc
        bq[j * P:(j + 1) * P] *= sc
    kv = {}
    vbias = {}
    for r, lnn, wn in [("o", "ln_other", "wkv_other"),
                       ("l", "ln_l2a", "wkv_l2a"),
                       ("g", "ln_g2a", "wkv_g2a")]:
        w, b = fold(g[lnn], g[wn])                # [128,256], [256]
        kv[r] = w
        kv[r + "_kb"] = b[:P]                      # k bias (zero for this model)
        vbias[r] = b[P:]                           # v bias -> folded into fc bias
    # fc weights / biases in device order (self, other, l2a, g2a), with the
    # v-bias fold: relu((o + vb) @ W + b) = relu(o @ W + (vb @ W + b))
    wfc = np.concatenate([g["self_fc"][0], g["fc_other"][0], g["fc_l2a"][0],
                          g["fc_g2a"][0]], axis=1)         # [128, 4*128]
    bfc = np.stack([
        g["self_fc"][1],
        g["fc_other"][1] + vbias["o"] @ g["fc_other"][0],
        g["fc_l2a"][1] + vbias["l"] @ g["fc_l2a"][0],
        g["fc_g2a"][1] + vbias["g"] @ g["fc_g2a"][0]], axis=1)
    # out_fc: cat order [self, other, l2a, g2a] rows of wout
    wout, bout = g["out_fc"]                       # [512,128],[128]
    wout_p = wout.reshape(4, P, P).transpose(1, 0, 2).reshape(P, 4 * P)
    # ffn
    w1, b1 = g["ffn_w1"]
    w2, b2 = g["ffn_w2"]
    w3, b3 = g["ffn_w3"]
    gam, bet = g["ffn_ln"]
    w1f = gam[:, None] * w1
    b1f = bet @ w1 + b1
    w3f = gam[:, None] * w3
    b3f = bet @ w3 + b3
    w2_p = w2.reshape(4, P, P).transpose(1, 0, 2).reshape(P, 4 * P)

    wpack = np.zeros((P, W_COLS), np.float32)
    wpack[:, _OFF["wq"]:_OFF["wq"] + 384] = wq
    for r, key in [("o", "kv_o"), ("l", "kv_l"), ("g", "kv_g")]:
        wpack[:, _OFF[key]:_OFF[key] + 256] = kv[r]
    wpack[:, _OFF["wfc"]:_OFF["wfc"] + 512] = wfc
    wpack[:, _OFF["wout"]:_OFF["wout"] + 512] = wout_p
    wpack[:, _OFF["w1"]:_OFF["w1"] + 512] = w1f
    wpack[:, _OFF["w3"]:_OFF["w3"] + 512] = w3f
    wpack[:, _OFF["w2"]:_OFF["w2"] + 512] = w2_p

    bpack = np.zeros((P, B_COLS), np.float32)
    bpack[:, _BOFF["bq"]:_BOFF["bq"] + 3] = bq.reshape(3, P).T
    bpack[:, _BOFF["bfc"]:_BOFF["bfc"] + 4] = bfc
    bpack[:, _BOFF["bout"]] = bout
    bpack[:, _BOFF["b1"]:_BOFF["b1"] + 4] = b1f.reshape(4, P).T
    bpack[:, _BOFF["b3"]:_BOFF["b3"] + 4] = b3f.reshape(4, P).T
    bpack[:, _BOFF["b2"]] = b2

    import ml_dtypes
    return wpack.astype(ml_dtypes.bfloat16), bpack


def _prep_mask(mask):
    """bool [Bl, NA, NK] -> notmask^T bf16 [Bl, 128, T*NA] (k-partition-major)."""
    import ml_dtypes
    bl, na, nk = mask.shape
    t = nk // P
    nmT = (~np.asarray(mask, bool)).astype(np.float32).transpose(0, 2, 1)
    nmT = nmT.reshape(bl, t, P, na).transpose(0, 2, 1, 3).reshape(bl, P, t * na)
    return np.ascontiguousarray(nmT.astype(ml_dtypes.bfloat16))


def kernel(agent_h, lane_h, poly_h, a2a_mask, l2a_mask, g2a_mask, params):
    agent_h = np.ascontiguousarray(np.asarray(agent_h, np.float32))
    lane_h = np.ascontiguousarray(np.asarray(lane_h, np.float32))
    poly_h = np.ascontiguousarray(np.asarray(poly_h, np.float32))
    wpack, bpack = _prep_host(params)
    nm_a = _prep_mask(np.asarray(a2a_mask))
    nm_l = _prep_mask(np.asarray(l2a_mask))
    nm_g = _prep_mask(np.asarray(g2a_mask))

    nc = _get_program()
    in_maps = []
    for c in range(N_CORES):
        sl = slice(c * B_LOC, (c + 1) * B_LOC)
        in_maps.append({
            "agent": agent_h[sl], "lane": lane_h[sl], "poly": poly_h[sl],
            "nm_a": nm_a[sl], "nm_l": nm_l[sl], "nm_g": nm_g[sl],
            "wpack": wpack, "bpack": bpack,
        })
    trace = bool(int(os.environ.get("KERNEL_TRACE", "0")))
    res = run_bass_kernel_spmd(nc, in_maps, list(range(N_CORES)), trace=trace)
    if trace and res.exec_time_ns is not None:
        print(f"HW exec time: {res.exec_time_ns} ns")
    out = np.concatenate([res.results[c]["out"] for c in range(N_CORES)], axis=0)
    return out.astype(np.float32)
